# revision 68
# baseline (speedup 1.0000x reference)
"""Trainium2 Bass kernel for nn_Attention_LR_65249143160949 (cross-attention block).

Sharding: 8 cores = 4 batches x 2 token-halves (1152 tokens each). Each core
computes k/v for its whole batch (cheap MQA single head, duplicated within the
pair) and q/attention/output for its own tokens. The host permutes tokens so
each core's own rows come first -> identical SPMD program, no collectives.

On-chip layout: features on partitions, tokens on the free axis. LayerNorm is
folded into the projections (pre-scaled weights + rank-1 -colsum*mu term).
Attention runs in sim^T layout (keys on partitions, query tokens free): kT is
rs-scaled so softmax is a plain exp; the denominator comes free as a ones
column (col 64) of the 128-col-padded v stationary (row 64 of the out psum).

Engine balance (the v1 kernel was ACT+PE serialized at ~460us; this one
measures ~320us):
- exp is split ACT (exact, bf16 out) / DVE (Schraudolph bit-trick: i16 =
  round(sim*184.665 + 16250.4) bitcast bf16, ~3% max err on weights).
- all fp32 broadcast/stat matmuls use float32r (1 cyc/row vs 4) or bf16.
- the 24 per-head reciprocals are batched: den rows DMA-gathered to one
  [8, 512] tile, ONE reciprocal per chunk, then per-pair selector matmuls
  (K=8 one-hot-rows, f32r) broadcast 1/den to the right partitions.
- head-PAIR stacking: the odd head's attn.v uses a second v stationary
  (v in cols 64:128, ones at col 32) so its output lands on psum
  partitions 64:128; ot tiles stack both heads -> out-proj runs K=128
  with pair-stacked Wout (half the matmuls), one rb-bcast per pair.
- attn.v lags exp by ALAG=4 j-tiles so DVE-queue latency never stalls PE.
- the tail (recip/norm/proj/LN2/y) is shredded into small thunks drained
  between head-pair passes of the NEXT chunk.
- GPSIMD (cannot touch PSUM) takes SBUF-only elementwise work: big
  memsets, LN2 squares, the residual add.
- v/wkv stationaries are 128-col padded/fused so FWL weight loads stay
  fast; a PE warmup chain pre-ramps the clock during the DMA wait.
- x arrives bf16 from the host, DMA'd in token chunks on the second
  (ACT) hwdge queue so LN1 starts ~12us in; fp32 x (residual only,
  own half only) goes last. Host-mirrored big-line DMA layouts were
  tried and are SLOWER (chip-level power throttle) - keep the small
  rearranged descriptors.

Walrus quirks handled: one sync-wait per TPB instruction (_split_multi_waits),
no custom DVE ops, engine ops must start at partition 0/32/64/96, GPSIMD has
no PSUM access, f32r tiles must be produced by a rounding op (DVE copy), and
row-tile-CONCURRENT matmul pairs must write different PSUM banks (same-bank
concurrent writes hard-crash the device).
"""

import sys

import numpy as np

if "/opt/trn_rl_repo" not in sys.path:
    sys.path.insert(0, "/opt/trn_rl_repo")

C = 512          # channels
N = 2304         # tokens per batch (48*48)
NH = 1152        # tokens per core
HEADS = 8
DH = 64
CTXL = 77
CTXD = 768
JT = 19          # j tiles of 128: 18 img + 1 (ctx 0:77 | null 77 | pad)
JP = JT * 128
CHUNKS = [(0, 512), (512, 512), (1024, 128)]  # (start, len) token chunks
NCH = len(CHUNKS)
KT = 4           # C / 128
EPS = 1e-5

FEXP_S = 184.6649186888274   # 128 / ln(2)
FEXP_C = 16250.4             # 127*128 - 5.6 (minimax-tuned, round-to-nearest)

PROFILE = False
PROFILE_DIR = None

_cached = {}


USE_DVE_EXP = True
USE_GPS_TAIL = True
USE_F32R = True
USE_DENT_DMA = True
ALAG = 4         # attn.v lags exp by this many j-tiles (512-token chunks)


def _exp_engine(cc, hg, jt):
    """Engine for the softmax exp of (chunk cc, head-pair hg, j-tile jt).
    512-token chunks: every 4th j-tile on DVE (bit-trick exp). Returns
    'act' or 'dve'."""
    if not USE_DVE_EXP:
        return "act"
    return "dve" if (jt % 5) == 4 else "act"


def _exp_engine_128(hg, grp):
    """Engine for the packed 4-j-tile exp groups of the 128-token chunk."""
    if not USE_DVE_EXP:
        return "act"
    return "dve" if grp in (1, 3) else "act"


def _split_multi_waits(nc):
    """Walrus codegen supports one sync-wait per TPB instruction (the EVENTS
    struct has a single wait slot). Tile attaches several. Split the extras
    onto same-engine NoOps inserted just before each instruction."""
    import concourse.mybir as mybir

    n = 0
    for fn in nc.m.functions:
        for bb in fn.blocks:
            insts = bb.instructions
            i = 0
            while i < len(insts):
                ins = insts[i]
                si = getattr(ins, "sync_info", None)
                if si is not None and si.on_wait and len(si.on_wait) > 1:
                    waits = list(si.on_wait)
                    for w in waits[:-1]:
                        n += 1
                        nop = mybir.InstNoOp(name=f"WSPLIT-{n}", engine=ins.engine)
                        nop.sync_info = mybir.SyncInfo(on_wait=[w], on_update=[])
                        insts.insert(i, nop)
                        i += 1
                    ins.sync_info = mybir.SyncInfo(
                        on_wait=[waits[-1]], on_update=si.on_update)
                i += 1
    return n


def _build_bass():
    import concourse.bass as bass
    import concourse.mybir as mybir
    import concourse.tile as tile
    from concourse.masks import make_identity
    from contextlib import ExitStack

    F32 = mybir.dt.float32
    F32R = mybir.dt.float32r if USE_F32R else mybir.dt.float32
    BF = mybir.dt.bfloat16
    I16 = mybir.dt.int16
    AF = mybir.ActivationFunctionType
    ALU = mybir.AluOpType

    nc = bass.Bass()
    xbf = nc.declare_dram_parameter("xbf", [C, N], BF, isOutput=False)
    x_own = nc.declare_dram_parameter("x_own", [C, NH], F32, isOutput=False)
    ctxt = nc.declare_dram_parameter("ctxt", [CTXL, CTXD], F32, isOutput=False)
    wq = nc.declare_dram_parameter("wq", [C, C], BF, isOutput=False)
    negcq = nc.declare_dram_parameter("negcq", [1, C], BF, isOutput=False)
    wkvc = nc.declare_dram_parameter("wkvc", [C, 128], BF, isOutput=False)
    ncskv = nc.declare_dram_parameter("ncskv", [1, 128], BF, isOutput=False)
    wctx = nc.declare_dram_parameter("wctx", [CTXD, 2 * DH], F32, isOutput=False)
    bctxk = nc.declare_dram_parameter("bctxk", [DH, 1], F32, isOutput=False)
    bctxv = nc.declare_dram_parameter("bctxv", [DH, 1], F32, isOutput=False)
    nullkt = nc.declare_dram_parameter("nullkt", [DH, 1], F32, isOutput=False)
    nullv = nc.declare_dram_parameter("nullv", [DH, 1], F32, isOutput=False)
    wout = nc.declare_dram_parameter("wout", [128, (HEADS // 2) * C], BF,
                                     isOutput=False)
    selin = nc.declare_dram_parameter("selin", [8, 8 * DH], F32, isOutput=False)
    outgr = nc.declare_dram_parameter("outgr", [1, C], F32, isOutput=False)
    y = nc.declare_dram_parameter("y", [C, NH], F32, isOutput=True)

    with tile.TileContext(nc) as tc, ExitStack() as ctx:
        pconst = ctx.enter_context(tc.tile_pool(name="const", bufs=1))
        pbig = ctx.enter_context(tc.tile_pool(name="big", bufs=1))

        ident = pconst.tile([128, 128], F32)
        make_identity(nc, ident[:])
        ident_bf = pconst.tile([128, 128], BF)
        make_identity(nc, ident_bf[:])
        ones_col = pconst.tile([128, 1], F32)
        nc.vector.memset(ones_col[:], 1.0)
        ones_col_bf = pconst.tile([128, 1], BF)
        nc.vector.memset(ones_col_bf[:], 1.0)
        ones_f = pconst.tile([1, 128], F32)
        nc.vector.memset(ones_f[:], 1.0)
        ones_r = pconst.tile([1, 128], F32R)
        nc.vector.tensor_copy(ones_r[:], ones_f[:])
        eps_col = pconst.tile([128, 1], F32)
        nc.vector.memset(eps_col[:], EPS)
        ones_blk = pconst.tile([128, 64], F32)
        nc.vector.memset(ones_blk[:], 1.0)
        sel_f = pconst.tile([8, 8 * DH], F32)
        sel = pconst.tile([8, 8 * DH], F32R)
        outgr_f = pconst.tile([1, C], F32)
        outgr_r = pconst.tile([1, C], F32R)

        x_sb = pbig.tile([128, KT * NH], F32)        # kt-major; OWN half only
        x_bf = pbig.tile([128, 6 * KT * 384], BF)   # chunk-major: [ch][kt][384] (kept)
        qT = pbig.tile([128, (HEADS // 2) * NH], BF)  # head-pair blocks
        kT2 = pbig.tile([128, JP], BF)               # rs-scaled keys, both halves
        v_sb = pbig.tile([128, JT * 128], BF)        # per j-tile [v 0:64|ones 64|pad]
        v_sb2 = pbig.tile([128, JT * 128], BF)       # odd-head: [0|ones@32|0|v 64:128]
        projBF = pbig.tile([128, KT * 512], BF)      # bf16 proj (per chunk)
        stats = pbig.tile([128, 40], F32)            # col jt: rs_j (v scaling)
        wout_sb = pbig.tile([128, (HEADS // 2) * C], BF)  # head-pair stacked
        # per-token stat rows on partition 0: mu 0:N | rs N:2N
        # (LN2 reuses per cc: mu2 at cc*CH, rs2 at N+cc*CH, ex2 at 2N+cc*CH)
        rows = pbig.tile([1, 2 * N + NH], F32)
        rows_bf = pbig.tile([1, N], BF)
        rows_r = pbig.tile([1, N], F32R)             # f32r copy of rs row
        R_RS, R_SC = N, 2 * N

        nc.sync.dma_start(sel_f[:], selin[:, :])
        nc.sync.dma_start(outgr_f[:], outgr[:, :])
        nc.vector.tensor_copy(sel[:], sel_f[:])
        nc.vector.tensor_copy(outgr_r[:], outgr_f[:])
        nc.sync.dma_start(wout_sb[:], wout[:, :])

        with tc.tile_pool(name="load", bufs=1) as pload, \
             tc.tile_pool(name="x2p", bufs=2) as px2, \
             tc.tile_pool(name="pss", bufs=2, space="PSUM") as pss:
            # psum tags: b1 [<=64,384]x2, bS [128,<=512]x4, bT [128,128]x2
            wq_sb = pload.tile([128, KT * C], BF)
            wkv_sb = pload.tile([128, KT * 128], BF)
            wctx_sb = pload.tile([128, CTXD], F32)
            negcq_sb = pload.tile([1, C], BF)
            ncskv_sb = pload.tile([1, 128], BF)
            bctxk_sb = pload.tile([DH, 1], F32)
            bctxv_sb = pload.tile([DH, 1], F32)
            vT = pload.tile([64, N], BF)
            ck_sb = pload.tile([64, CTXL], F32)
            cv_sb = pload.tile([64, CTXL + 1], F32)
            nullk_st = pload.tile([DH, 1], F32)
            nullv_st = pload.tile([DH, 1], F32)
            ctx_sb = pload.tile([CTXL, CTXD], F32)
            ctxnT = pload.tile([128, 6 * CTXL], F32)
            ex2 = pload.tile([1, N], F32)
            kk = pload.tile([128, 512], F32)

            # PE warmup: ~4us of back-to-back junk matmuls while the DMAs
            # stream in, so the tensor engine reaches its fast pstate
            # before LN1's first real matmul.
            ps_wu = pss.tile([128, 128], F32, tag="bT")
            for _ in range(6):
                nc.tensor.matmul(ps_wu[:, :], ones_f[:], ones_f[:],
                                 start=True, stop=True)

            # x_bf chunks on the ACT queue so LN1 starts while the SP
            # queue streams ctx/weights. SBUF x_bf is chunk-major.
            xbf_d = xbf[:].rearrange("(k p) n -> p k n", p=128)
            xv4 = x_bf[:].rearrange("p (c k n) -> p c k n", c=6, k=KT)
            for ch in range(6):
                a, b = ch * 384, (ch + 1) * 384
                q = nc.scalar if ch % 2 == 0 else nc.sync
                q.dma_start(xv4[:, ch, :, :], xbf_d[:, :, a:b])
            nc.sync.dma_start(ctx_sb[:], ctxt[:, :])
            nc.sync.dma_start(wctx_sb[:].rearrange("p (k n) -> p k n", k=6),
                              wctx[:].rearrange("(k p) n -> p k n", p=128))
            nc.sync.dma_start(bctxk_sb[:], bctxk[:, :])
            nc.sync.dma_start(bctxv_sb[:], bctxv[:, :])
            nc.sync.dma_start(nullk_st[:], nullkt[:, :])
            nc.sync.dma_start(nullv_st[:], nullv[:, :])
            nc.sync.dma_start(wkv_sb[:].rearrange("p (k n) -> p k n", k=KT),
                              wkvc[:].rearrange("(k p) n -> p k n", p=128))
            nc.sync.dma_start(ncskv_sb[:], ncskv[:, :])
            nc.scalar.dma_start(wq_sb[:].rearrange("p (k n) -> p k n", k=KT),
                                wq[:].rearrange("(k p) n -> p k n", p=128))
            nc.sync.dma_start(negcq_sb[:], negcq[:, :])
            x_v = x_sb[:].rearrange("p (k n) -> p k n", k=KT)
            nc.sync.dma_start(x_v[:, :, 0:NH],
                              x_own[:].rearrange("(k p) n -> p k n", p=128))

            # ---- context: LN (layout A, bn_stats) + k/v projection ----
            cstat = pload.tile([CTXL, 3, 6], F32)
            for sg in range(3):
                nc.vector.bn_stats(cstat[:, sg, :],
                                   ctx_sb[:, sg * 256 : (sg + 1) * 256])
            cmv = pload.tile([CTXL, 2], F32)
            nc.vector.bn_aggr(cmv[:], cstat[:])
            nc.scalar.activation(cmv[:, 1:2], cmv[:, 1:2], AF.Ln,
                                 bias=eps_col[0:CTXL, :])
            nc.scalar.activation(cmv[:, 1:2], cmv[:, 1:2], AF.Exp, scale=-0.5)
            nc.vector.tensor_scalar(
                out=ctx_sb[:], in0=ctx_sb[:],
                scalar1=cmv[:, 0:1], scalar2=cmv[:, 1:2],
                op0=ALU.subtract, op1=ALU.mult)
            for kt in range(6):
                ps_ct = pss.tile([128, 128], F32, tag="bT")
                nc.tensor.transpose(ps_ct[:, 0:CTXL],
                                    ctx_sb[:, kt * 128 : (kt + 1) * 128],
                                    ident[:CTXL, :CTXL])
                nc.vector.tensor_copy(ctxnT[:, kt * CTXL : (kt + 1) * CTXL],
                                      ps_ct[:, 0:CTXL])
            ps_ck = pss.tile([64, 384], F32, tag="b1")
            ps_cv = pss.tile([64, 384], F32, tag="b1")
            for kt in range(6):
                nc.tensor.matmul(ps_ck[:, 0:CTXL],
                                 wctx_sb[:, kt * 128 : kt * 128 + DH],
                                 ctxnT[:, kt * CTXL : (kt + 1) * CTXL],
                                 start=(kt == 0), stop=(kt == 5))
                nc.tensor.matmul(ps_cv[:, 0:CTXL],
                                 wctx_sb[:, kt * 128 + DH : (kt + 1) * 128],
                                 ctxnT[:, kt * CTXL : (kt + 1) * CTXL],
                                 start=(kt == 0), stop=(kt == 5))
            nc.vector.tensor_scalar_add(ck_sb[:], ps_ck[:, 0:CTXL], bctxk_sb[:])
            nc.vector.tensor_scalar_add(cv_sb[:, 0:CTXL], ps_cv[:, 0:CTXL],
                                        bctxv_sb[:])
            nc.vector.tensor_copy(cv_sb[:, CTXL : CTXL + 1], nullv_st[:])

            # ---- fused per-384-token-chunk pipeline:
            # LN1 stats -> kv projection -> v tiles for the 3 j-tiles,
            # software-pipelined with the x_bf chunk DMAs.
            nc.gpsimd.memset(v_sb[:], 0.0)
            nc.gpsimd.memset(v_sb2[:], 0.0)
            nc.gpsimd.memset(kT2[0:64, 18 * 128 : JP], 0.0)
            for ch in range(6):
                sl = slice(ch * 384, (ch + 1) * 384)
                ps_r1 = pss.tile([64, 384], F32, tag="b1")
                for kt in range(KT):
                    xo = (ch * KT + kt) * 384
                    nc.tensor.matmul(
                        ps_r1[0:1, :], ones_col_bf[:],
                        x_bf[:, xo : xo + 384],
                        start=(kt == 0), stop=(kt == KT - 1))
                nc.scalar.mul(rows[0:1, sl], ps_r1[0:1, :], 1.0 / C)
                nc.vector.tensor_copy(rows_bf[0:1, sl], rows[0:1, sl])
                x2 = px2.tile([128, KT * 384], BF, tag="x2")
                ps_r2 = pss.tile([64, 384], F32, tag="b1")
                for kt in range(KT):
                    xs = x_bf[:, (ch * KT + kt) * 384 : (ch * KT + kt + 1) * 384]
                    nc.vector.tensor_mul(x2[:, kt * 384 : (kt + 1) * 384], xs, xs)
                    nc.tensor.matmul(
                        ps_r2[0:1, :], ones_col_bf[:],
                        x2[:, kt * 384 : (kt + 1) * 384],
                        start=(kt == 0), stop=(kt == KT - 1))
                nc.scalar.mul(ex2[0:1, ch * 384 : (ch + 1) * 384],
                              ps_r2[0:1, :], 1.0 / C)
                a, b = R_RS + ch * 384, R_RS + (ch + 1) * 384
                mu = rows[0:1, ch * 384 : (ch + 1) * 384]
                nc.vector.tensor_mul(rows[0:1, a:b], mu, mu)
                nc.vector.tensor_sub(rows[0:1, a:b],
                                     ex2[0:1, ch * 384 : (ch + 1) * 384],
                                     rows[0:1, a:b])
                nc.scalar.activation(rows[0:1, a:b], rows[0:1, a:b], AF.Ln,
                                     bias=eps_col[0:1, :])
                nc.scalar.activation(rows[0:1, a:b], rows[0:1, a:b], AF.Exp,
                                     scale=-0.5)
                nc.vector.tensor_copy(rows_r[0:1, sl], rows[0:1, a:b])
                # kv projection for this chunk (LN folded; k -> kT2 top half)
                kvl = 384
                ps_kv = pss.tile([128, 512], F32, tag="bS")
                for kt in range(KT):
                    xs = x_bf[:, (ch * KT + kt) * 384 : (ch * KT + kt + 1) * 384]
                    nc.tensor.matmul(ps_kv[:, 0:kvl],
                                     wkv_sb[:, kt * 128 : (kt + 1) * 128],
                                     xs, start=(kt == 0), stop=False)
                nc.tensor.matmul(ps_kv[:, 0:kvl], ncskv_sb[:], rows_bf[0:1, sl],
                                 start=False, stop=True)
                ps_bc = pss.tile([128, 512], F32, tag="bS")
                nc.tensor.matmul(ps_bc[:, 0:kvl], ones_r[0:1, 0:128],
                                 rows_r[0:1, sl])
                nc.vector.tensor_copy(kk[64:128, 0:kvl], ps_kv[64:128, 0:kvl])
                nc.vector.tensor_mul(kT2[64:128, sl], kk[64:128, 0:kvl],
                                     ps_bc[64:128, 0:kvl])
                nc.vector.tensor_copy(vT[:, sl], ps_kv[0:64, 0:kvl])
                # v tiles + rs columns for the 3 j-tiles of this chunk
                for jt in range(3 * ch, 3 * ch + 3):
                    ps_c = pss.tile([128, 128], F32, tag="bT")
                    nc.tensor.matmul(
                        ps_c[:, 0:1],
                        rows[0:1, R_RS + jt * 128 : R_RS + (jt + 1) * 128],
                        ones_col[0:1, :])
                    nc.vector.tensor_copy(stats[:, jt : jt + 1], ps_c[:, 0:1])
                    ps_vt = pss.tile([128, 128], BF, tag="bT")
                    nc.tensor.transpose(ps_vt[:, 0:64],
                                        vT[:, jt * 128 : (jt + 1) * 128],
                                        ident_bf[:64, :64])
                    vb = jt * 128
                    nc.vector.tensor_scalar_mul(v_sb[:, vb : vb + DH],
                                                ps_vt[:, 0:64],
                                                stats[:, jt : jt + 1])
                    nc.vector.memset(v_sb[:, vb + DH : vb + DH + 1], 1.0)
                    nc.vector.tensor_scalar_mul(v_sb2[:, vb + 64 : vb + 128],
                                                ps_vt[:, 0:64],
                                                stats[:, jt : jt + 1])
                    nc.vector.memset(v_sb2[:, vb + 32 : vb + 33], 1.0)

            # ---- j-tile 18: [ctx 0:77 | null 77 | pad 78:128] ----
            nc.vector.tensor_copy(kT2[0:64, 18 * 128 : 18 * 128 + CTXL], ck_sb[:])
            nc.vector.tensor_copy(kT2[0:64, 18 * 128 + CTXL : 18 * 128 + CTXL + 1],
                                  nullk_st[:])
            VB = 18 * 128
            ps_cvt = pss.tile([128, 128], F32, tag="bT")
            nc.tensor.transpose(ps_cvt[0 : CTXL + 1, 0:64], cv_sb[:],
                                ident[:64, :64])
            nc.vector.tensor_copy(v_sb[0 : CTXL + 1, VB : VB + DH],
                                  ps_cvt[0 : CTXL + 1, 0:64])
            nc.vector.memset(v_sb[0 : CTXL + 1, VB + DH : VB + DH + 1], 1.0)
            nc.vector.tensor_copy(v_sb2[0 : CTXL + 1, VB + 64 : VB + 128],
                                  ps_cvt[0 : CTXL + 1, 0:64])
            nc.vector.memset(v_sb2[0 : CTXL + 1, VB + 32 : VB + 33], 1.0)

            # ---- duplicate kT to partitions 0:64 (sbuf->sbuf DMA) ----
            nc.sync.dma_start(kT2[0:64, 0 : 18 * 128], kT2[64:128, 0 : 18 * 128])
            nc.sync.dma_start(kT2[64:128, 18 * 128 : JP],
                              kT2[0:64, 18 * 128 : JP])

            # ---- q projection (head pairs; LN + 1/sqrt(dh) folded) ----
            for a0, ln in CHUNKS:
                sl = slice(a0, a0 + ln)
                ps_rs = pss.tile([128, 512], F32, tag="bS")
                nc.tensor.matmul(ps_rs[:, 0:ln], ones_r[0:1, :],
                                 rows_r[0:1, sl])
                rs_b = px2.tile([128, 512], F32, tag="rsb")
                nc.vector.tensor_copy(rs_b[:, 0:ln], ps_rs[:, 0:ln])
                for hg in range(HEADS // 2):
                    ps_q = pss.tile([128, 512], F32, tag="bS")
                    for kt in range(KT):
                        nc.tensor.matmul(
                            ps_q[:, 0:ln],
                            wq_sb[:, kt * C + hg * 128 : kt * C + (hg + 1) * 128],
                            x_bf[:, kt * N + a0 : kt * N + a0 + ln],
                            start=(kt == 0), stop=False)
                    nc.tensor.matmul(ps_q[:, 0:ln],
                                     negcq_sb[0:1, hg * 128 : (hg + 1) * 128],
                                     rows_bf[0:1, sl], start=False, stop=True)
                    nc.vector.tensor_mul(
                        qT[:, hg * NH + a0 : hg * NH + a0 + ln],
                        ps_q[:, 0:ln], rs_b[:, 0:ln])

        # ========= attention + output + LN2 + residual, per chunk =========
        # Per (chunk, head-pair): sim pair (row-tiled concurrent) -> exp on
        # ACT or DVE (bit-trick) -> attn.v pair lagging one j-tile. The tail
        # (recip batch, norm, out-proj, LN2, y) is deferred one stage so it
        # never head-of-line-blocks the PE queue.
        with tc.tile_pool(name="attb", bufs=6) as patb, \
             tc.tile_pool(name="atti", bufs=3) as pati, \
             tc.tile_pool(name="outp", bufs=6) as pout, \
             tc.tile_pool(name="pocp", bufs=6) as ppoc, \
             tc.tile_pool(name="denp", bufs=2) as pden, \
             tc.tile_pool(name="recp", bufs=2) as prec, \
             tc.tile_pool(name="rowp", bufs=1) as prow, \
             tc.tile_pool(name="p2p", bufs=1) as pp2, \
             tc.tile_pool(name="yp", bufs=2) as pyt, \
             tc.tile_pool(name="psatt", bufs=2, space="PSUM") as psA, \
             tc.tile_pool(name="psacc", bufs=4, space="PSUM") as psB:

            def at_tile(engine):
                if engine == "act":
                    return patb.tile([128, 1024], BF, tag="atb", name="atb")
                return pati.tile([128, 1024], I16, tag="ati", name="ati")

            def do_exp(engine, at, ps_s, lo, hi):
                if engine == "act":
                    nc.scalar.activation(at[:, lo:hi], ps_s[:, lo:hi], AF.Exp)
                else:
                    nc.vector.tensor_scalar(
                        out=at[:, lo:hi], in0=ps_s[:, lo:hi],
                        scalar1=FEXP_S, scalar2=FEXP_C,
                        op0=ALU.mult, op1=ALU.add)

            def at_slice(at_eng, lo, hi):
                at, eng = at_eng
                ap = at[:, lo:hi]
                return ap if eng == "act" else ap.bitcast(BF)

            def run_hg_512(cc, hg, a0, ln, dent, pocs, workq):
                po0 = psB.tile([128, 512], F32, tag="pop", bufs=2)
                po1 = psB.tile([128, 512], F32, tag="pop", bufs=2)
                po = [po0, po1]
                q0 = qT[0:64, hg * NH + a0 : hg * NH + a0 + ln]
                q1 = qT[64:128, hg * NH + a0 : hg * NH + a0 + ln]
                ats = [None] * JT
                for jt in range(JT):
                    ps_s = psA.tile([128, 1024], F32, tag="sim")
                    nc.tensor.matmul(ps_s[:, 0:ln],
                                     kT2[0:64, jt * 128 : (jt + 1) * 128],
                                     q0, start=True, stop=True)
                    nc.tensor.matmul(ps_s[:, 512 : 512 + ln],
                                     kT2[64:128, jt * 128 : (jt + 1) * 128],
                                     q1, start=True, stop=True)
                    eng = _exp_engine(cc, hg, jt)
                    at = at_tile(eng)
                    do_exp(eng, at, ps_s, 0, 1024)
                    ats[jt] = (at, eng)
                    if jt >= ALAG:
                        j0 = jt - ALAG
                        nc.tensor.matmul(po[0][:, 0:ln],
                                         v_sb[:, j0 * 128 : (j0 + 1) * 128],
                                         at_slice(ats[j0], 0, ln),
                                         start=(j0 == 0), stop=False)
                        nc.tensor.matmul(po[1][:, 0:ln],
                                         v_sb2[:, j0 * 128 : (j0 + 1) * 128],
                                         at_slice(ats[j0], 512, 512 + ln),
                                         start=(j0 == 0), stop=False)
                        ats[j0] = None
                for j0 in range(JT - ALAG, JT):
                    nc.tensor.matmul(po[0][:, 0:ln],
                                     v_sb[:, j0 * 128 : (j0 + 1) * 128],
                                     at_slice(ats[j0], 0, ln),
                                     start=(j0 == 0), stop=(j0 == JT - 1))
                    nc.tensor.matmul(po[1][:, 0:ln],
                                     v_sb2[:, j0 * 128 : (j0 + 1) * 128],
                                     at_slice(ats[j0], 512, 512 + ln),
                                     start=(j0 == 0), stop=(j0 == JT - 1))
                _extract(hg, ln, po, dent, pocs)

            def run_hg_128(cc, hg, a0, ln, dent, pocs, workq):
                # 19 j-tiles packed 4-per-psum-tile; slot g: q0 at col 128g
                # (bank A), q1 at 512+128g (bank B) so the row-tiled
                # concurrent sim pair never co-writes one psum bank.
                po0 = psB.tile([128, 512], F32, tag="pop", bufs=2)
                po1 = psB.tile([128, 512], F32, tag="pop", bufs=2)
                po = [po0, po1]
                q0 = qT[0:64, hg * NH + a0 : hg * NH + a0 + ln]
                q1 = qT[64:128, hg * NH + a0 : hg * NH + a0 + ln]
                GRP = [(0, 4), (4, 4), (8, 4), (12, 4), (16, 3)]
                ats = [None] * len(GRP)
                for gi, (jt0, ng) in enumerate(GRP):
                    ps_s = psA.tile([128, 1024], F32, tag="sim")
                    for g in range(ng):
                        jt = jt0 + g
                        nc.tensor.matmul(
                            ps_s[:, 128 * g : 128 * g + ln],
                            kT2[0:64, jt * 128 : (jt + 1) * 128],
                            q0, start=True, stop=True)
                        nc.tensor.matmul(
                            ps_s[:, 512 + 128 * g : 512 + 128 * g + ln],
                            kT2[64:128, jt * 128 : (jt + 1) * 128],
                            q1, start=True, stop=True)
                    eng = _exp_engine_128(hg, gi)
                    at = at_tile(eng)
                    if ng == 4:
                        do_exp(eng, at, ps_s, 0, 1024)
                    else:
                        do_exp(eng, at, ps_s, 0, 128 * ng)
                        do_exp(eng, at, ps_s, 512, 512 + 128 * ng)
                    ats[gi] = (at, eng)
                    if gi > 0:
                        _attnv_128(po, ats[gi - 1], GRP[gi - 1], ln,
                                   start=(gi == 1), stop=False)
                        ats[gi - 1] = None
                _attnv_128(po, ats[-1], GRP[-1], ln, start=False, stop=True)
                _extract(hg, ln, po, dent, pocs)

            def _attnv_128(po, at_eng, grp, ln, start, stop):
                jt0, ng = grp
                for g in range(ng):
                    jt = jt0 + g
                    last = stop and (g == ng - 1)
                    nc.tensor.matmul(po[0][:, 0:ln],
                                     v_sb[:, jt * 128 : (jt + 1) * 128],
                                     at_slice(at_eng, 128 * g, 128 * g + ln),
                                     start=(start and g == 0), stop=last)
                    nc.tensor.matmul(po[1][:, 0:ln],
                                     v_sb2[:, jt * 128 : (jt + 1) * 128],
                                     at_slice(at_eng, 512 + 128 * g,
                                              512 + 128 * g + ln),
                                     start=(start and g == 0), stop=last)

            def _extract(hg, ln, po, dent, pocs):
                # stacked ot: even head out -> partitions 0:64 (from po0),
                # odd head out -> 64:128 (po1; its v sat cols 64:128).
                # dens: po0 row 64 (even), po1 row 32 (odd).
                otp = ppoc.tile([128, 512], BF, tag="poc")
                nc.vector.tensor_copy(otp[0:64, 0:ln], po[0][0:64, 0:ln])
                nc.vector.tensor_copy(otp[64:128, 0:ln], po[1][64:128, 0:ln])
                stg = ppoc.tile([65, 512], BF, tag="stg")
                nc.vector.tensor_copy(stg[64:65, 0:ln], po[0][64:65, 0:ln])
                nc.vector.tensor_copy(stg[32:33, 0:ln], po[1][32:33, 0:ln])
                nc.sync.dma_start(dent[2 * hg : 2 * hg + 1, 0:ln],
                                  stg[64:65, 0:ln])
                nc.sync.dma_start(dent[2 * hg + 1 : 2 * hg + 2, 0:ln],
                                  stg[32:33, 0:ln])
                pocs.append(otp)

            def tail_phases(cc, a0, ln, dent, pocs):
                ots = []
                rec8r = prec.tile([8, 512], F32R, tag="rec")
                rs2r = prow.tile([1, 2 * 512], F32R, tag="r2")

                def th_recip():
                    dentf = prec.tile([8, 512], F32, tag="dentf")
                    nc.vector.tensor_copy(dentf[:, 0:ln], dent[:, 0:ln])
                    rec8 = prec.tile([8, 512], F32, tag="rec8")
                    nc.vector.reciprocal(rec8[:, 0:ln], dentf[:, 0:ln])
                    nc.vector.tensor_copy(rec8r[:, 0:ln], rec8[:, 0:ln])

                def norm_pair(hg):
                    def th():
                        ps_rb = psB.tile([128, 512], F32, tag="po", bufs=2)
                        nc.tensor.matmul(ps_rb[:, 0:ln],
                                         sel[:, hg * 128 : (hg + 1) * 128],
                                         rec8r[:, 0:ln])
                        ot = pout.tile([128, 512], BF, tag="ot", name="ot")
                        nc.vector.tensor_mul(ot[:, 0:ln], pocs[hg][:, 0:ln],
                                             ps_rb[:, 0:ln])
                        ots.append(ot)
                    return th

                def proj_ct(ct):
                    def th():
                        ps_p = psB.tile([128, 512], F32, tag="po", bufs=2)
                        for hg in range(HEADS // 2):
                            nc.tensor.matmul(
                                ps_p[:, 0:ln],
                                wout_sb[:, hg * C + ct * 128 : hg * C + (ct + 1) * 128],
                                ots[hg][:, 0:ln],
                                start=(hg == 0), stop=(hg == HEADS // 2 - 1))
                        nc.vector.tensor_copy(
                            projBF[:, ct * 512 : ct * 512 + ln],
                            ps_p[:, 0:ln])
                    return th

                def th_mean():
                    ps_m2 = psB.tile([128, 512], F32, tag="po", bufs=2)
                    for ct in range(KT):
                        nc.tensor.matmul(ps_m2[0:1, 0:ln], ones_col_bf[:],
                                         projBF[:, ct * 512 : ct * 512 + ln],
                                         start=(ct == 0), stop=(ct == KT - 1))
                    nc.scalar.mul(rows[0:1, a0 : a0 + ln], ps_m2[0:1, 0:ln],
                                  1.0 / C)

                def th_var():
                    sca, scb = R_SC + a0, R_SC + a0 + ln
                    p2 = pp2.tile([128, KT * 512], BF, tag="p2")
                    ps_q2 = psB.tile([128, 512], F32, tag="po", bufs=2)
                    for ct in range(KT):
                        pslc = projBF[:, ct * 512 : ct * 512 + ln]
                        eng2 = nc.gpsimd if USE_GPS_TAIL else nc.vector
                        eng2.tensor_mul(p2[:, ct * 512 : ct * 512 + ln],
                                        pslc, pslc)
                        nc.tensor.matmul(ps_q2[0:1, 0:ln], ones_col_bf[:],
                                         p2[:, ct * 512 : ct * 512 + ln],
                                         start=(ct == 0), stop=(ct == KT - 1))
                    nc.scalar.mul(rows[0:1, sca:scb], ps_q2[0:1, 0:ln], 1.0 / C)

                def th_rows():
                    ra, rb2 = R_RS + a0, R_RS + a0 + ln
                    sca, scb = R_SC + a0, R_SC + a0 + ln
                    nc.vector.tensor_mul(rows[0:1, ra:rb2],
                                         rows[0:1, a0 : a0 + ln],
                                         rows[0:1, a0 : a0 + ln])
                    nc.vector.tensor_sub(rows[0:1, ra:rb2], rows[0:1, sca:scb],
                                         rows[0:1, ra:rb2])
                    nc.scalar.activation(rows[0:1, ra:rb2], rows[0:1, ra:rb2],
                                         AF.Ln, bias=eps_col[0:1, :])
                    nc.scalar.activation(rows[0:1, ra:rb2], rows[0:1, ra:rb2],
                                         AF.Exp, scale=-0.5)
                    nc.vector.tensor_mul(rows[0:1, sca:scb],
                                         rows[0:1, a0 : a0 + ln],
                                         rows[0:1, ra:rb2])
                    nc.vector.tensor_copy(rs2r[0:1, 0:ln], rows[0:1, ra:rb2])
                    nc.vector.tensor_copy(rs2r[0:1, 512 : 512 + ln],
                                          rows[0:1, sca:scb])

                def y_ct(ct):
                    def th():
                        ps_gb = psB.tile([128, 512], F32, tag="po", bufs=2)
                        gsl = outgr_r[0:1, ct * 128 : (ct + 1) * 128]
                        nc.tensor.matmul(ps_gb[:, 0:ln], gsl, rs2r[0:1, 0:ln])
                        yt = pyt.tile([128, 512], F32, tag="yt", name="yt")
                        pslice = projBF[:, ct * 512 : ct * 512 + ln]
                        nc.vector.tensor_mul(yt[:, 0:ln], pslice, ps_gb[:, 0:ln])
                        ps_gm = psB.tile([128, 512], F32, tag="po", bufs=2)
                        nc.tensor.matmul(ps_gm[:, 0:ln], gsl,
                                         rs2r[0:1, 512 : 512 + ln])
                        nc.vector.tensor_sub(yt[:, 0:ln], yt[:, 0:ln],
                                             ps_gm[:, 0:ln])
                        eng3 = nc.gpsimd if USE_GPS_TAIL else nc.vector
                        eng3.tensor_add(
                            yt[:, 0:ln], yt[:, 0:ln],
                            x_sb[:, ct * NH + a0 : ct * NH + a0 + ln])
                        nc.sync.dma_start(
                            y[ct * 128 : (ct + 1) * 128, a0 : a0 + ln],
                            yt[:, 0:ln])
                    return th

                return ([th_recip] + [norm_pair(hg) for hg in range(4)]
                        + [proj_ct(ct) for ct in range(KT)]
                        + [th_mean, th_var, th_rows]
                        + [y_ct(ct) for ct in range(KT)])

            workq = []
            for cc, (a0, ln) in enumerate(CHUNKS):
                pocs = []
                dent = pden.tile([8, 512], BF, tag="dent", name="dent")
                GSZ = [3, 2, 4, 7]
                for hg in range(HEADS // 2):
                    if ln == 512:
                        run_hg_512(cc, hg, a0, ln, dent, pocs, workq)
                    else:
                        run_hg_128(cc, hg, a0, ln, dent, pocs, workq)
                    for _ in range(GSZ[hg]):
                        if workq:
                            workq.pop(0)()
                workq.extend(tail_phases(cc, a0, ln, dent, pocs))
            for th in workq:
                th()
    _split_multi_waits(nc)
    return nc


def _prep_inputs(x, context, norm_gamma, null_kv, Wq, Wkv, ctx_ln_g, ctx_ln_b,
                 Wctx, bctx, Wout, out_ln_g):
    import ml_dtypes
    bf = ml_dtypes.bfloat16
    f = np.float32
    x = np.asarray(x, f).reshape(4, C, N)
    context = np.asarray(context, f)
    g = np.asarray(norm_gamma, f)
    scale = 1.0 / np.sqrt(DH)
    wq_h = (g[:, None] * np.asarray(Wq, f)) * scale
    negcq_h = -wq_h.sum(0, dtype=np.float64).astype(f)[None, :]
    wkv_h = g[:, None] * np.asarray(Wkv, f)
    # combined stationary: [v | k] so k lands on psum rows 64:128
    wkvc_h = np.concatenate([wkv_h[:, DH:], wkv_h[:, :DH]], axis=1)
    ncs = -wkv_h.sum(0, dtype=np.float64).astype(f)
    ncskv_h = np.concatenate([ncs[DH:], ncs[:DH]])[None, :]
    wctx_h = np.asarray(ctx_ln_g, f)[:, None] * np.asarray(Wctx, f)
    bctx_h = (np.asarray(bctx, f) + np.asarray(ctx_ln_b, f) @ np.asarray(Wctx, f))
    null = np.asarray(null_kv, f)
    W_o = np.asarray(Wout, f)
    # head-pair stacked: rows 0:64 = even head dims, 64:128 = odd head dims
    wout_b = np.concatenate(
        [np.concatenate([W_o[2 * hg * DH : (2 * hg + 1) * DH, :],
                         W_o[(2 * hg + 1) * DH : (2 * hg + 2) * DH, :]], axis=0)
         for hg in range(HEADS // 2)], axis=1)
    # pair selector: block hg, cols 0:64 -> row 2hg, cols 64:128 -> row 2hg+1
    sel_h = np.zeros((8, 4 * 128), f)
    for hg in range(4):
        sel_h[2 * hg, hg * 128 : hg * 128 + 64] = 1.0
        sel_h[2 * hg + 1, hg * 128 + 64 : (hg + 1) * 128] = 1.0

    def mirror(a):
        """[K*128, n] row-major -> SBUF mirror [128, K*n]."""
        K = a.shape[0] // 128
        return np.ascontiguousarray(
            a.reshape(K, 128, -1).transpose(1, 0, 2).reshape(128, -1))

    shared = {
        "wq": np.ascontiguousarray(wq_h).astype(bf),
        "negcq": negcq_h.astype(bf),
        "wkvc": np.ascontiguousarray(wkvc_h).astype(bf),
        "ncskv": np.ascontiguousarray(ncskv_h).astype(bf),
        "wctx": np.ascontiguousarray(wctx_h),
        "bctxk": np.ascontiguousarray(bctx_h[:DH, None]),
        "bctxv": np.ascontiguousarray(bctx_h[DH:, None]),
        "nullkt": np.ascontiguousarray(null[0][:, None]),
        "nullv": np.ascontiguousarray(null[1][:, None]),
        "wout": np.ascontiguousarray(wout_b).astype(bf),
        "selin": sel_h,
        "outgr": np.ascontiguousarray(np.asarray(out_ln_g, f)[None, :]),
    }
    in_maps = []
    for core in range(8):
        b, half = core // 2, core % 2
        m = dict(shared)
        xo = x[b][:, half * NH : (half + 1) * NH]
        xt = x[b][:, (1 - half) * NH : (2 - half) * NH]
        m["x_own"] = np.ascontiguousarray(xo)
        m["xbf"] = np.ascontiguousarray(
            np.concatenate([xo, xt], axis=1)).astype(bf)
        m["ctxt"] = np.ascontiguousarray(context[b])
        in_maps.append(m)
    return in_maps


_LDW_OPT = [False]


def _patch_ldw_opt():
    import concourse.bass_utils as bu
    if getattr(bu, "_ldwopt_patched", False):
        return
    orig = bu.run_command

    def run2(cmd, **kw):
        if _LDW_OPT[0]:
            cmd = [c.replace("--enable-ldw-opt=false", "--enable-ldw-opt=true")
                   for c in cmd]
        return orig(cmd, **kw)

    bu.run_command = run2
    bu._ldwopt_patched = True


def kernel(**inputs):
    from concourse.bass_utils import run_bass_kernel_spmd
    _patch_ldw_opt()

    if "nc" not in _cached:
        _cached["nc"] = _build_bass()
    nc = _cached["nc"]
    in_maps = _prep_inputs(**inputs)
    kw = {}
    if PROFILE:
        import importlib.util

        if "antenv.axon_hooks" not in sys.modules:
            spec = importlib.util.spec_from_file_location(
                "antenv.axon_hooks", "/opt/trn_rl_repo/antenv/axon_hooks.py")
            m = importlib.util.module_from_spec(spec)
            spec.loader.exec_module(m)
            sys.modules["antenv.axon_hooks"] = m
            import antenv

            antenv.axon_hooks = m
        kw = dict(trace=True, tmpdir=PROFILE_DIR)
    res = run_bass_kernel_spmd(nc, in_maps, list(range(8)), **kw)
    _cached["last"] = res
    out = np.empty((4, C, N), np.float32)
    for core in range(8):
        b, half = core // 2, core % 2
        out[b][:, half * NH : (half + 1) * NH] = res.results[core]["y"]
    return out.reshape(4, C, 48, 48)


# revision 69
# speedup vs baseline: 1.0450x; 1.0450x over previous
"""Trainium2 Bass kernel for nn_Attention_LR_65249143160949 (cross-attention block).

Sharding: 8 cores = 4 batches x 2 token-halves (1152 tokens each). Each core
computes k/v for its whole batch (cheap MQA single head, duplicated within the
pair) and q/attention/output for its own tokens. The host permutes tokens so
each core's own rows come first -> identical SPMD program, no collectives.

On-chip layout: features on partitions, tokens on the free axis. LayerNorm is
folded into the projections (pre-scaled weights + rank-1 -colsum*mu term).
Attention runs in sim^T layout (keys on partitions, query tokens free): kT is
rs-scaled so softmax is a plain exp; the denominator comes free as a ones
column (col 64) of the 128-col-padded v stationary (row 64 of the out psum).

Engine balance (the v1 kernel was ACT+PE serialized at ~460us; this one
measures ~320us):
- exp is split ACT (exact, bf16 out) / DVE (Schraudolph bit-trick: i16 =
  round(sim*184.665 + 16250.4) bitcast bf16, ~3% max err on weights).
- all fp32 broadcast/stat matmuls use float32r (1 cyc/row vs 4) or bf16.
- the 24 per-head reciprocals are batched: den rows DMA-gathered to one
  [8, 512] tile, ONE reciprocal per chunk, then per-pair selector matmuls
  (K=8 one-hot-rows, f32r) broadcast 1/den to the right partitions.
- head-PAIR stacking: the odd head's attn.v uses a second v stationary
  (v in cols 64:128, ones at col 32) so its output lands on psum
  partitions 64:128; ot tiles stack both heads -> out-proj runs K=128
  with pair-stacked Wout (half the matmuls), one rb-bcast per pair.
- attn.v lags exp by ALAG=4 j-tiles so DVE-queue latency never stalls PE.
- the tail (recip/norm/proj/LN2/y) is shredded into small thunks drained
  between head-pair passes of the NEXT chunk.
- GPSIMD (cannot touch PSUM) takes SBUF-only elementwise work: big
  memsets, LN2 squares, the residual add.
- v/wkv stationaries are 128-col padded/fused so FWL weight loads stay
  fast; a PE warmup chain pre-ramps the clock during the DMA wait.
- x arrives bf16 from the host, DMA'd in token chunks on the second
  (ACT) hwdge queue so LN1 starts ~12us in; fp32 x (residual only,
  own half only) goes last. Host-mirrored big-line DMA layouts were
  tried and are SLOWER (chip-level power throttle) - keep the small
  rearranged descriptors.

Walrus quirks handled: one sync-wait per TPB instruction (_split_multi_waits),
no custom DVE ops, engine ops must start at partition 0/32/64/96, GPSIMD has
no PSUM access, f32r tiles must be produced by a rounding op (DVE copy), and
row-tile-CONCURRENT matmul pairs must write different PSUM banks (same-bank
concurrent writes hard-crash the device).
"""

import sys

import numpy as np

if "/opt/trn_rl_repo" not in sys.path:
    sys.path.insert(0, "/opt/trn_rl_repo")

C = 512          # channels
N = 2304         # tokens per batch (48*48)
NH = 1152        # tokens per core
HEADS = 8
DH = 64
CTXL = 77
CTXD = 768
JT = 19          # j tiles of 128: 18 img + 1 (ctx 0:77 | null 77 | pad)
JP = JT * 128
CHUNKS = [(0, 512), (512, 512), (1024, 128)]  # (start, len) token chunks
NCH = len(CHUNKS)
KT = 4           # C / 128
EPS = 1e-5

FEXP_S = 184.6649186888274   # 128 / ln(2)
FEXP_C = 16250.4             # 127*128 - 5.6 (minimax-tuned, round-to-nearest)

PROFILE = False
PROFILE_DIR = None

_cached = {}


USE_DVE_EXP = True
USE_GPS_TAIL = True
USE_F32R = True
USE_DENT_DMA = True
ALAG = 4         # attn.v lags exp by this many j-tiles (512-token chunks)


def _exp_engine(cc, hg, jt):
    """Engine for the softmax exp of (chunk cc, head-pair hg, j-tile jt).
    512-token chunks: every 4th j-tile on DVE (bit-trick exp). Returns
    'act' or 'dve'."""
    if not USE_DVE_EXP:
        return "act"
    return "dve" if (jt % 5) == 4 else "act"


def _exp_engine_128(hg, grp):
    """Engine for the packed 4-j-tile exp groups of the 128-token chunk."""
    if not USE_DVE_EXP:
        return "act"
    return "dve" if grp in (1, 3) else "act"


def _split_multi_waits(nc):
    """Walrus codegen supports one sync-wait per TPB instruction (the EVENTS
    struct has a single wait slot). Tile attaches several. Split the extras
    onto same-engine NoOps inserted just before each instruction."""
    import concourse.mybir as mybir

    n = 0
    for fn in nc.m.functions:
        for bb in fn.blocks:
            insts = bb.instructions
            i = 0
            while i < len(insts):
                ins = insts[i]
                si = getattr(ins, "sync_info", None)
                if si is not None and si.on_wait and len(si.on_wait) > 1:
                    waits = list(si.on_wait)
                    for w in waits[:-1]:
                        n += 1
                        nop = mybir.InstNoOp(name=f"WSPLIT-{n}", engine=ins.engine)
                        nop.sync_info = mybir.SyncInfo(on_wait=[w], on_update=[])
                        insts.insert(i, nop)
                        i += 1
                    ins.sync_info = mybir.SyncInfo(
                        on_wait=[waits[-1]], on_update=si.on_update)
                i += 1
    return n


def _build_bass():
    import concourse.bass as bass
    import concourse.mybir as mybir
    import concourse.tile as tile
    from concourse.masks import make_identity
    from contextlib import ExitStack

    F32 = mybir.dt.float32
    F32R = mybir.dt.float32r if USE_F32R else mybir.dt.float32
    BF = mybir.dt.bfloat16
    I16 = mybir.dt.int16
    AF = mybir.ActivationFunctionType
    ALU = mybir.AluOpType

    nc = bass.Bass()
    xbf = nc.declare_dram_parameter("xbf", [C, N], BF, isOutput=False)
    x_own = nc.declare_dram_parameter("x_own", [C, NH], F32, isOutput=False)
    ctxt = nc.declare_dram_parameter("ctxt", [CTXL, CTXD], F32, isOutput=False)
    wq = nc.declare_dram_parameter("wq", [C, C], BF, isOutput=False)
    negcq = nc.declare_dram_parameter("negcq", [1, C], BF, isOutput=False)
    wkvc = nc.declare_dram_parameter("wkvc", [C, 128], BF, isOutput=False)
    ncskv = nc.declare_dram_parameter("ncskv", [1, 128], BF, isOutput=False)
    wctx = nc.declare_dram_parameter("wctx", [CTXD, 2 * DH], F32, isOutput=False)
    bctxk = nc.declare_dram_parameter("bctxk", [DH, 1], F32, isOutput=False)
    bctxv = nc.declare_dram_parameter("bctxv", [DH, 1], F32, isOutput=False)
    nullkt = nc.declare_dram_parameter("nullkt", [DH, 1], F32, isOutput=False)
    nullv = nc.declare_dram_parameter("nullv", [DH, 1], F32, isOutput=False)
    wout = nc.declare_dram_parameter("wout", [128, (HEADS // 2) * C], BF,
                                     isOutput=False)
    selin = nc.declare_dram_parameter("selin", [8, 8 * DH], F32, isOutput=False)
    outgr = nc.declare_dram_parameter("outgr", [1, C], F32, isOutput=False)
    y = nc.declare_dram_parameter("y", [C, NH], F32, isOutput=True)

    with tile.TileContext(nc) as tc, ExitStack() as ctx:
        pconst = ctx.enter_context(tc.tile_pool(name="const", bufs=1))
        pbig = ctx.enter_context(tc.tile_pool(name="big", bufs=1))

        ident = pconst.tile([128, 128], F32)
        make_identity(nc, ident[:])
        ident_bf = pconst.tile([128, 128], BF)
        make_identity(nc, ident_bf[:])
        ones_col = pconst.tile([128, 1], F32)
        nc.vector.memset(ones_col[:], 1.0)
        ones_col_bf = pconst.tile([128, 1], BF)
        nc.vector.memset(ones_col_bf[:], 1.0)
        ones_f = pconst.tile([1, 128], F32)
        nc.vector.memset(ones_f[:], 1.0)
        ones_r = pconst.tile([1, 128], F32R)
        nc.vector.tensor_copy(ones_r[:], ones_f[:])
        eps_col = pconst.tile([128, 1], F32)
        nc.vector.memset(eps_col[:], EPS)
        ones_blk = pconst.tile([128, 64], F32)
        nc.vector.memset(ones_blk[:], 1.0)
        sel_f = pconst.tile([8, 8 * DH], F32)
        sel = pconst.tile([8, 8 * DH], F32R)
        outgr_f = pconst.tile([1, C], F32)
        outgr_r = pconst.tile([1, C], F32R)

        x_sb = pbig.tile([128, KT * NH], F32)        # kt-major; OWN half only
        x_bf = pbig.tile([128, 6 * KT * 384], BF)   # chunk-major: [ch][kt][384] (kept)
        qT = pbig.tile([128, (HEADS // 2) * NH], BF)  # head-pair blocks
        kT2 = pbig.tile([128, JP], BF)               # rs-scaled keys, both halves
        v_sb = pbig.tile([128, JT * 128], BF)        # per j-tile [v 0:64|ones 64|pad]
        v_sb2 = pbig.tile([128, JT * 128], BF)       # odd-head: [0|ones@32|0|v 64:128]
        projBF = pbig.tile([128, KT * 512], BF)      # bf16 proj (per chunk)
        stats = pbig.tile([128, 40], F32)            # col jt: rs_j (v scaling)
        wout_sb = pbig.tile([128, (HEADS // 2) * C], BF)  # head-pair stacked
        # per-token stat rows on partition 0: mu 0:N | rs N:2N
        # (LN2 reuses per cc: mu2 at cc*CH, rs2 at N+cc*CH, ex2 at 2N+cc*CH)
        rows = pbig.tile([1, 2 * N + NH], F32)
        rows_bf = pbig.tile([1, N], BF)
        rows_r = pbig.tile([1, N], F32R)             # f32r copy of rs row
        R_RS, R_SC = N, 2 * N

        nc.sync.dma_start(sel_f[:], selin[:, :])
        nc.sync.dma_start(outgr_f[:], outgr[:, :])
        nc.vector.tensor_copy(sel[:], sel_f[:])
        nc.vector.tensor_copy(outgr_r[:], outgr_f[:])
        nc.sync.dma_start(wout_sb[:], wout[:, :])

        with tc.tile_pool(name="load", bufs=1) as pload, \
             tc.tile_pool(name="x2p", bufs=2) as px2, \
             tc.tile_pool(name="pss", bufs=2, space="PSUM") as pss:
            # psum tags: b1 [<=64,384]x2, bS [128,<=512]x4, bT [128,128]x2
            wq_sb = pload.tile([128, KT * C], BF)
            wkv_sb = pload.tile([128, KT * 128], BF)
            wctx_sb = pload.tile([128, CTXD], F32)
            negcq_sb = pload.tile([1, C], BF)
            ncskv_sb = pload.tile([1, 128], BF)
            bctxk_sb = pload.tile([DH, 1], F32)
            bctxv_sb = pload.tile([DH, 1], F32)
            vT = pload.tile([64, N], BF)
            ck_sb = pload.tile([64, CTXL], F32)
            cv_sb = pload.tile([64, CTXL + 1], F32)
            nullk_st = pload.tile([DH, 1], F32)
            nullv_st = pload.tile([DH, 1], F32)
            ctx_sb = pload.tile([CTXL, CTXD], F32)
            ctxnT = pload.tile([128, 6 * CTXL], F32)
            ex2 = pload.tile([1, N], F32)
            kk = pload.tile([128, 512], F32)

            # PE warmup: ~4us of back-to-back junk matmuls while the DMAs
            # stream in, so the tensor engine reaches its fast pstate
            # before LN1's first real matmul.
            ps_wu = pss.tile([128, 128], F32, tag="bT")
            for _ in range(6):
                nc.tensor.matmul(ps_wu[:, :], ones_f[:], ones_f[:],
                                 start=True, stop=True)

            # x_bf chunks on the ACT queue so LN1 starts while the SP
            # queue streams ctx/weights. SBUF x_bf is chunk-major.
            xbf_d = xbf[:].rearrange("(k p) n -> p k n", p=128)
            xv4 = x_bf[:].rearrange("p (c k n) -> p c k n", c=6, k=KT)
            for ch in range(6):
                a, b = ch * 384, (ch + 1) * 384
                q = nc.scalar if ch % 2 == 0 else nc.sync
                q.dma_start(xv4[:, ch, :, :], xbf_d[:, :, a:b])
            nc.sync.dma_start(ctx_sb[:], ctxt[:, :])
            nc.sync.dma_start(wctx_sb[:].rearrange("p (k n) -> p k n", k=6),
                              wctx[:].rearrange("(k p) n -> p k n", p=128))
            nc.sync.dma_start(bctxk_sb[:], bctxk[:, :])
            nc.sync.dma_start(bctxv_sb[:], bctxv[:, :])
            nc.sync.dma_start(nullk_st[:], nullkt[:, :])
            nc.sync.dma_start(nullv_st[:], nullv[:, :])
            nc.sync.dma_start(wkv_sb[:].rearrange("p (k n) -> p k n", k=KT),
                              wkvc[:].rearrange("(k p) n -> p k n", p=128))
            nc.sync.dma_start(ncskv_sb[:], ncskv[:, :])
            nc.scalar.dma_start(wq_sb[:].rearrange("p (k n) -> p k n", k=KT),
                                wq[:].rearrange("(k p) n -> p k n", p=128))
            nc.sync.dma_start(negcq_sb[:], negcq[:, :])
            x_v = x_sb[:].rearrange("p (k n) -> p k n", k=KT)
            nc.sync.dma_start(x_v[:, :, 0:NH],
                              x_own[:].rearrange("(k p) n -> p k n", p=128))

            # ---- fused per-384-token-chunk pipeline:
            # LN1 stats -> kv projection -> v tiles for the 3 j-tiles,
            # software-pipelined with the x_bf chunk DMAs.
            nc.gpsimd.memset(v_sb[:], 0.0)
            nc.gpsimd.memset(v_sb2[:], 0.0)
            nc.gpsimd.memset(kT2[0:64, 18 * 128 : JP], 0.0)
            for ch in range(6):
                sl = slice(ch * 384, (ch + 1) * 384)
                ps_r1 = pss.tile([64, 384], F32, tag="b1")
                for kt in range(KT):
                    xo = (ch * KT + kt) * 384
                    nc.tensor.matmul(
                        ps_r1[0:1, :], ones_col_bf[:],
                        x_bf[:, xo : xo + 384],
                        start=(kt == 0), stop=(kt == KT - 1))
                nc.scalar.mul(rows[0:1, sl], ps_r1[0:1, :], 1.0 / C)
                nc.vector.tensor_copy(rows_bf[0:1, sl], rows[0:1, sl])
                x2 = px2.tile([128, KT * 384], BF, tag="x2")
                ps_r2 = pss.tile([64, 384], F32, tag="b1")
                for kt in range(KT):
                    xs = x_bf[:, (ch * KT + kt) * 384 : (ch * KT + kt + 1) * 384]
                    nc.vector.tensor_mul(x2[:, kt * 384 : (kt + 1) * 384], xs, xs)
                    nc.tensor.matmul(
                        ps_r2[0:1, :], ones_col_bf[:],
                        x2[:, kt * 384 : (kt + 1) * 384],
                        start=(kt == 0), stop=(kt == KT - 1))
                nc.scalar.mul(ex2[0:1, ch * 384 : (ch + 1) * 384],
                              ps_r2[0:1, :], 1.0 / C)
                a, b = R_RS + ch * 384, R_RS + (ch + 1) * 384
                mu = rows[0:1, ch * 384 : (ch + 1) * 384]
                nc.vector.tensor_mul(rows[0:1, a:b], mu, mu)
                nc.vector.tensor_sub(rows[0:1, a:b],
                                     ex2[0:1, ch * 384 : (ch + 1) * 384],
                                     rows[0:1, a:b])
                nc.scalar.activation(rows[0:1, a:b], rows[0:1, a:b], AF.Ln,
                                     bias=eps_col[0:1, :])
                nc.scalar.activation(rows[0:1, a:b], rows[0:1, a:b], AF.Exp,
                                     scale=-0.5)
                nc.vector.tensor_copy(rows_r[0:1, sl], rows[0:1, a:b])
                # kv projection for this chunk (LN folded; k -> kT2 top half)
                kvl = 384
                ps_kv = pss.tile([128, 512], F32, tag="bS")
                for kt in range(KT):
                    xs = x_bf[:, (ch * KT + kt) * 384 : (ch * KT + kt + 1) * 384]
                    nc.tensor.matmul(ps_kv[:, 0:kvl],
                                     wkv_sb[:, kt * 128 : (kt + 1) * 128],
                                     xs, start=(kt == 0), stop=False)
                nc.tensor.matmul(ps_kv[:, 0:kvl], ncskv_sb[:], rows_bf[0:1, sl],
                                 start=False, stop=True)
                ps_bc = pss.tile([128, 512], F32, tag="bS")
                nc.tensor.matmul(ps_bc[:, 0:kvl], ones_r[0:1, 0:128],
                                 rows_r[0:1, sl])
                nc.vector.tensor_copy(kk[64:128, 0:kvl], ps_kv[64:128, 0:kvl])
                nc.vector.tensor_mul(kT2[64:128, sl], kk[64:128, 0:kvl],
                                     ps_bc[64:128, 0:kvl])
                nc.vector.tensor_copy(vT[:, sl], ps_kv[0:64, 0:kvl])
                # v tiles + rs columns for the 3 j-tiles of this chunk
                for jt in range(3 * ch, 3 * ch + 3):
                    ps_c = pss.tile([128, 128], F32, tag="bT")
                    nc.tensor.matmul(
                        ps_c[:, 0:1],
                        rows[0:1, R_RS + jt * 128 : R_RS + (jt + 1) * 128],
                        ones_col[0:1, :])
                    nc.vector.tensor_copy(stats[:, jt : jt + 1], ps_c[:, 0:1])
                    ps_vt = pss.tile([128, 128], BF, tag="bT")
                    nc.tensor.transpose(ps_vt[:, 0:64],
                                        vT[:, jt * 128 : (jt + 1) * 128],
                                        ident_bf[:64, :64])
                    vb = jt * 128
                    nc.vector.tensor_scalar_mul(v_sb[:, vb : vb + DH],
                                                ps_vt[:, 0:64],
                                                stats[:, jt : jt + 1])
                    nc.vector.memset(v_sb[:, vb + DH : vb + DH + 1], 1.0)
                    nc.vector.tensor_scalar_mul(v_sb2[:, vb + 64 : vb + 128],
                                                ps_vt[:, 0:64],
                                                stats[:, jt : jt + 1])
                    nc.vector.memset(v_sb2[:, vb + 32 : vb + 33], 1.0)

            # ---- context: LN (layout A, bn_stats) + k/v projection ----
            cstat = pload.tile([CTXL, 3, 6], F32)
            for sg in range(3):
                nc.vector.bn_stats(cstat[:, sg, :],
                                   ctx_sb[:, sg * 256 : (sg + 1) * 256])
            cmv = pload.tile([CTXL, 2], F32)
            nc.vector.bn_aggr(cmv[:], cstat[:])
            nc.scalar.activation(cmv[:, 1:2], cmv[:, 1:2], AF.Ln,
                                 bias=eps_col[0:CTXL, :])
            nc.scalar.activation(cmv[:, 1:2], cmv[:, 1:2], AF.Exp, scale=-0.5)
            nc.vector.tensor_scalar(
                out=ctx_sb[:], in0=ctx_sb[:],
                scalar1=cmv[:, 0:1], scalar2=cmv[:, 1:2],
                op0=ALU.subtract, op1=ALU.mult)
            for kt in range(6):
                ps_ct = pss.tile([128, 128], F32, tag="bT")
                nc.tensor.transpose(ps_ct[:, 0:CTXL],
                                    ctx_sb[:, kt * 128 : (kt + 1) * 128],
                                    ident[:CTXL, :CTXL])
                nc.vector.tensor_copy(ctxnT[:, kt * CTXL : (kt + 1) * CTXL],
                                      ps_ct[:, 0:CTXL])
            ps_ck = pss.tile([64, 384], F32, tag="b1")
            ps_cv = pss.tile([64, 384], F32, tag="b1")
            for kt in range(6):
                nc.tensor.matmul(ps_ck[:, 0:CTXL],
                                 wctx_sb[:, kt * 128 : kt * 128 + DH],
                                 ctxnT[:, kt * CTXL : (kt + 1) * CTXL],
                                 start=(kt == 0), stop=(kt == 5))
                nc.tensor.matmul(ps_cv[:, 0:CTXL],
                                 wctx_sb[:, kt * 128 + DH : (kt + 1) * 128],
                                 ctxnT[:, kt * CTXL : (kt + 1) * CTXL],
                                 start=(kt == 0), stop=(kt == 5))
            nc.vector.tensor_scalar_add(ck_sb[:], ps_ck[:, 0:CTXL], bctxk_sb[:])
            nc.vector.tensor_scalar_add(cv_sb[:, 0:CTXL], ps_cv[:, 0:CTXL],
                                        bctxv_sb[:])
            nc.vector.tensor_copy(cv_sb[:, CTXL : CTXL + 1], nullv_st[:])

            # ---- j-tile 18: [ctx 0:77 | null 77 | pad 78:128] ----
            nc.vector.tensor_copy(kT2[0:64, 18 * 128 : 18 * 128 + CTXL], ck_sb[:])
            nc.vector.tensor_copy(kT2[0:64, 18 * 128 + CTXL : 18 * 128 + CTXL + 1],
                                  nullk_st[:])
            VB = 18 * 128
            ps_cvt = pss.tile([128, 128], F32, tag="bT")
            nc.tensor.transpose(ps_cvt[0 : CTXL + 1, 0:64], cv_sb[:],
                                ident[:64, :64])
            nc.vector.tensor_copy(v_sb[0 : CTXL + 1, VB : VB + DH],
                                  ps_cvt[0 : CTXL + 1, 0:64])
            nc.vector.memset(v_sb[0 : CTXL + 1, VB + DH : VB + DH + 1], 1.0)
            nc.vector.tensor_copy(v_sb2[0 : CTXL + 1, VB + 64 : VB + 128],
                                  ps_cvt[0 : CTXL + 1, 0:64])
            nc.vector.memset(v_sb2[0 : CTXL + 1, VB + 32 : VB + 33], 1.0)

            # ---- duplicate kT to partitions 0:64 (sbuf->sbuf DMA) ----
            nc.sync.dma_start(kT2[0:64, 0 : 18 * 128], kT2[64:128, 0 : 18 * 128])
            nc.sync.dma_start(kT2[64:128, 18 * 128 : JP],
                              kT2[0:64, 18 * 128 : JP])

            # ---- q projection (head pairs; LN + 1/sqrt(dh) folded) ----
            for a0, ln in CHUNKS:
                sl = slice(a0, a0 + ln)
                ps_rs = pss.tile([128, 512], F32, tag="bS")
                nc.tensor.matmul(ps_rs[:, 0:ln], ones_r[0:1, :],
                                 rows_r[0:1, sl])
                rs_b = px2.tile([128, 512], F32, tag="rsb")
                nc.vector.tensor_copy(rs_b[:, 0:ln], ps_rs[:, 0:ln])
                for hg in range(HEADS // 2):
                    ps_q = pss.tile([128, 512], F32, tag="bS")
                    for kt in range(KT):
                        nc.tensor.matmul(
                            ps_q[:, 0:ln],
                            wq_sb[:, kt * C + hg * 128 : kt * C + (hg + 1) * 128],
                            x_bf[:, kt * N + a0 : kt * N + a0 + ln],
                            start=(kt == 0), stop=False)
                    nc.tensor.matmul(ps_q[:, 0:ln],
                                     negcq_sb[0:1, hg * 128 : (hg + 1) * 128],
                                     rows_bf[0:1, sl], start=False, stop=True)
                    nc.vector.tensor_mul(
                        qT[:, hg * NH + a0 : hg * NH + a0 + ln],
                        ps_q[:, 0:ln], rs_b[:, 0:ln])

        # ========= attention + output + LN2 + residual, per chunk =========
        # Per (chunk, head-pair): sim pair (row-tiled concurrent) -> exp on
        # ACT or DVE (bit-trick) -> attn.v pair lagging one j-tile. The tail
        # (recip batch, norm, out-proj, LN2, y) is deferred one stage so it
        # never head-of-line-blocks the PE queue.
        with tc.tile_pool(name="attb", bufs=6) as patb, \
             tc.tile_pool(name="atti", bufs=3) as pati, \
             tc.tile_pool(name="outp", bufs=6) as pout, \
             tc.tile_pool(name="pocp", bufs=6) as ppoc, \
             tc.tile_pool(name="denp", bufs=2) as pden, \
             tc.tile_pool(name="recp", bufs=2) as prec, \
             tc.tile_pool(name="rowp", bufs=1) as prow, \
             tc.tile_pool(name="p2p", bufs=1) as pp2, \
             tc.tile_pool(name="yp", bufs=2) as pyt, \
             tc.tile_pool(name="psatt", bufs=2, space="PSUM") as psA, \
             tc.tile_pool(name="psacc", bufs=4, space="PSUM") as psB:

            def at_tile(engine):
                if engine == "act":
                    return patb.tile([128, 1024], BF, tag="atb", name="atb")
                return pati.tile([128, 1024], I16, tag="ati", name="ati")

            def do_exp(engine, at, ps_s, lo, hi):
                if engine == "act":
                    nc.scalar.activation(at[:, lo:hi], ps_s[:, lo:hi], AF.Exp)
                else:
                    nc.vector.tensor_scalar(
                        out=at[:, lo:hi], in0=ps_s[:, lo:hi],
                        scalar1=FEXP_S, scalar2=FEXP_C,
                        op0=ALU.mult, op1=ALU.add)

            def at_slice(at_eng, lo, hi):
                at, eng = at_eng
                ap = at[:, lo:hi]
                return ap if eng == "act" else ap.bitcast(BF)

            def run_hg_512(cc, hg, a0, ln, dent, pocs, workq):
                po0 = psB.tile([128, 512], F32, tag="pop", bufs=2)
                po1 = psB.tile([128, 512], F32, tag="pop", bufs=2)
                po = [po0, po1]
                q0 = qT[0:64, hg * NH + a0 : hg * NH + a0 + ln]
                q1 = qT[64:128, hg * NH + a0 : hg * NH + a0 + ln]
                ats = [None] * JT
                for jt in range(JT):
                    ps_s = psA.tile([128, 1024], F32, tag="sim")
                    nc.tensor.matmul(ps_s[:, 0:ln],
                                     kT2[0:64, jt * 128 : (jt + 1) * 128],
                                     q0, start=True, stop=True)
                    nc.tensor.matmul(ps_s[:, 512 : 512 + ln],
                                     kT2[64:128, jt * 128 : (jt + 1) * 128],
                                     q1, start=True, stop=True)
                    eng = _exp_engine(cc, hg, jt)
                    at = at_tile(eng)
                    do_exp(eng, at, ps_s, 0, 1024)
                    ats[jt] = (at, eng)
                    if jt >= ALAG:
                        j0 = jt - ALAG
                        nc.tensor.matmul(po[0][:, 0:ln],
                                         v_sb[:, j0 * 128 : (j0 + 1) * 128],
                                         at_slice(ats[j0], 0, ln),
                                         start=(j0 == 0), stop=False)
                        nc.tensor.matmul(po[1][:, 0:ln],
                                         v_sb2[:, j0 * 128 : (j0 + 1) * 128],
                                         at_slice(ats[j0], 512, 512 + ln),
                                         start=(j0 == 0), stop=False)
                        ats[j0] = None
                for j0 in range(JT - ALAG, JT):
                    nc.tensor.matmul(po[0][:, 0:ln],
                                     v_sb[:, j0 * 128 : (j0 + 1) * 128],
                                     at_slice(ats[j0], 0, ln),
                                     start=(j0 == 0), stop=(j0 == JT - 1))
                    nc.tensor.matmul(po[1][:, 0:ln],
                                     v_sb2[:, j0 * 128 : (j0 + 1) * 128],
                                     at_slice(ats[j0], 512, 512 + ln),
                                     start=(j0 == 0), stop=(j0 == JT - 1))
                _extract(hg, ln, po, dent, pocs)

            def run_hg_128(cc, hg, a0, ln, dent, pocs, workq):
                # 19 j-tiles packed 4-per-psum-tile; slot g: q0 at col 128g
                # (bank A), q1 at 512+128g (bank B) so the row-tiled
                # concurrent sim pair never co-writes one psum bank.
                po0 = psB.tile([128, 512], F32, tag="pop", bufs=2)
                po1 = psB.tile([128, 512], F32, tag="pop", bufs=2)
                po = [po0, po1]
                q0 = qT[0:64, hg * NH + a0 : hg * NH + a0 + ln]
                q1 = qT[64:128, hg * NH + a0 : hg * NH + a0 + ln]
                GRP = [(0, 4), (4, 4), (8, 4), (12, 4), (16, 3)]
                ats = [None] * len(GRP)
                for gi, (jt0, ng) in enumerate(GRP):
                    ps_s = psA.tile([128, 1024], F32, tag="sim")
                    for g in range(ng):
                        jt = jt0 + g
                        nc.tensor.matmul(
                            ps_s[:, 128 * g : 128 * g + ln],
                            kT2[0:64, jt * 128 : (jt + 1) * 128],
                            q0, start=True, stop=True)
                        nc.tensor.matmul(
                            ps_s[:, 512 + 128 * g : 512 + 128 * g + ln],
                            kT2[64:128, jt * 128 : (jt + 1) * 128],
                            q1, start=True, stop=True)
                    eng = _exp_engine_128(hg, gi)
                    at = at_tile(eng)
                    if ng == 4:
                        do_exp(eng, at, ps_s, 0, 1024)
                    else:
                        do_exp(eng, at, ps_s, 0, 128 * ng)
                        do_exp(eng, at, ps_s, 512, 512 + 128 * ng)
                    ats[gi] = (at, eng)
                    if gi > 0:
                        _attnv_128(po, ats[gi - 1], GRP[gi - 1], ln,
                                   start=(gi == 1), stop=False)
                        ats[gi - 1] = None
                _attnv_128(po, ats[-1], GRP[-1], ln, start=False, stop=True)
                _extract(hg, ln, po, dent, pocs)

            def _attnv_128(po, at_eng, grp, ln, start, stop):
                jt0, ng = grp
                for g in range(ng):
                    jt = jt0 + g
                    last = stop and (g == ng - 1)
                    nc.tensor.matmul(po[0][:, 0:ln],
                                     v_sb[:, jt * 128 : (jt + 1) * 128],
                                     at_slice(at_eng, 128 * g, 128 * g + ln),
                                     start=(start and g == 0), stop=last)
                    nc.tensor.matmul(po[1][:, 0:ln],
                                     v_sb2[:, jt * 128 : (jt + 1) * 128],
                                     at_slice(at_eng, 512 + 128 * g,
                                              512 + 128 * g + ln),
                                     start=(start and g == 0), stop=last)

            def _extract(hg, ln, po, dent, pocs):
                # stacked ot: even head out -> partitions 0:64 (from po0),
                # odd head out -> 64:128 (po1; its v sat cols 64:128).
                # dens: po0 row 64 (even), po1 row 32 (odd).
                otp = ppoc.tile([128, 512], BF, tag="poc")
                nc.vector.tensor_copy(otp[0:64, 0:ln], po[0][0:64, 0:ln])
                nc.vector.tensor_copy(otp[64:128, 0:ln], po[1][64:128, 0:ln])
                stg = ppoc.tile([65, 512], BF, tag="stg")
                nc.vector.tensor_copy(stg[64:65, 0:ln], po[0][64:65, 0:ln])
                nc.vector.tensor_copy(stg[32:33, 0:ln], po[1][32:33, 0:ln])
                nc.sync.dma_start(dent[2 * hg : 2 * hg + 1, 0:ln],
                                  stg[64:65, 0:ln])
                nc.sync.dma_start(dent[2 * hg + 1 : 2 * hg + 2, 0:ln],
                                  stg[32:33, 0:ln])
                pocs.append(otp)

            def tail_phases(cc, a0, ln, dent, pocs):
                ots = []
                rec8r = prec.tile([8, 512], F32R, tag="rec")
                rs2r = prow.tile([1, 2 * 512], F32R, tag="r2")

                def th_recip():
                    dentf = prec.tile([8, 512], F32, tag="dentf")
                    nc.vector.tensor_copy(dentf[:, 0:ln], dent[:, 0:ln])
                    rec8 = prec.tile([8, 512], F32, tag="rec8")
                    nc.vector.reciprocal(rec8[:, 0:ln], dentf[:, 0:ln])
                    nc.vector.tensor_copy(rec8r[:, 0:ln], rec8[:, 0:ln])

                def norm_pair(hg):
                    def th():
                        ps_rb = psB.tile([128, 512], F32, tag="po", bufs=2)
                        nc.tensor.matmul(ps_rb[:, 0:ln],
                                         sel[:, hg * 128 : (hg + 1) * 128],
                                         rec8r[:, 0:ln])
                        ot = pout.tile([128, 512], BF, tag="ot", name="ot")
                        nc.vector.tensor_mul(ot[:, 0:ln], pocs[hg][:, 0:ln],
                                             ps_rb[:, 0:ln])
                        ots.append(ot)
                    return th

                def proj_ct(ct):
                    def th():
                        ps_p = psB.tile([128, 512], F32, tag="po", bufs=2)
                        for hg in range(HEADS // 2):
                            nc.tensor.matmul(
                                ps_p[:, 0:ln],
                                wout_sb[:, hg * C + ct * 128 : hg * C + (ct + 1) * 128],
                                ots[hg][:, 0:ln],
                                start=(hg == 0), stop=(hg == HEADS // 2 - 1))
                        nc.vector.tensor_copy(
                            projBF[:, ct * 512 : ct * 512 + ln],
                            ps_p[:, 0:ln])
                    return th

                def th_mean():
                    ps_m2 = psB.tile([128, 512], F32, tag="po", bufs=2)
                    for ct in range(KT):
                        nc.tensor.matmul(ps_m2[0:1, 0:ln], ones_col_bf[:],
                                         projBF[:, ct * 512 : ct * 512 + ln],
                                         start=(ct == 0), stop=(ct == KT - 1))
                    nc.scalar.mul(rows[0:1, a0 : a0 + ln], ps_m2[0:1, 0:ln],
                                  1.0 / C)

                def th_var():
                    sca, scb = R_SC + a0, R_SC + a0 + ln
                    p2 = pp2.tile([128, KT * 512], BF, tag="p2")
                    ps_q2 = psB.tile([128, 512], F32, tag="po", bufs=2)
                    for ct in range(KT):
                        pslc = projBF[:, ct * 512 : ct * 512 + ln]
                        eng2 = nc.gpsimd if USE_GPS_TAIL else nc.vector
                        eng2.tensor_mul(p2[:, ct * 512 : ct * 512 + ln],
                                        pslc, pslc)
                        nc.tensor.matmul(ps_q2[0:1, 0:ln], ones_col_bf[:],
                                         p2[:, ct * 512 : ct * 512 + ln],
                                         start=(ct == 0), stop=(ct == KT - 1))
                    nc.scalar.mul(rows[0:1, sca:scb], ps_q2[0:1, 0:ln], 1.0 / C)

                def th_rows():
                    ra, rb2 = R_RS + a0, R_RS + a0 + ln
                    sca, scb = R_SC + a0, R_SC + a0 + ln
                    nc.vector.tensor_mul(rows[0:1, ra:rb2],
                                         rows[0:1, a0 : a0 + ln],
                                         rows[0:1, a0 : a0 + ln])
                    nc.vector.tensor_sub(rows[0:1, ra:rb2], rows[0:1, sca:scb],
                                         rows[0:1, ra:rb2])
                    nc.scalar.activation(rows[0:1, ra:rb2], rows[0:1, ra:rb2],
                                         AF.Ln, bias=eps_col[0:1, :])
                    nc.scalar.activation(rows[0:1, ra:rb2], rows[0:1, ra:rb2],
                                         AF.Exp, scale=-0.5)
                    nc.vector.tensor_mul(rows[0:1, sca:scb],
                                         rows[0:1, a0 : a0 + ln],
                                         rows[0:1, ra:rb2])
                    nc.vector.tensor_copy(rs2r[0:1, 0:ln], rows[0:1, ra:rb2])
                    nc.vector.tensor_copy(rs2r[0:1, 512 : 512 + ln],
                                          rows[0:1, sca:scb])

                def y_ct(ct):
                    def th():
                        ps_gb = psB.tile([128, 512], F32, tag="po", bufs=2)
                        gsl = outgr_r[0:1, ct * 128 : (ct + 1) * 128]
                        nc.tensor.matmul(ps_gb[:, 0:ln], gsl, rs2r[0:1, 0:ln])
                        yt = pyt.tile([128, 512], F32, tag="yt", name="yt")
                        pslice = projBF[:, ct * 512 : ct * 512 + ln]
                        nc.vector.tensor_mul(yt[:, 0:ln], pslice, ps_gb[:, 0:ln])
                        ps_gm = psB.tile([128, 512], F32, tag="po", bufs=2)
                        nc.tensor.matmul(ps_gm[:, 0:ln], gsl,
                                         rs2r[0:1, 512 : 512 + ln])
                        nc.vector.tensor_sub(yt[:, 0:ln], yt[:, 0:ln],
                                             ps_gm[:, 0:ln])
                        eng3 = nc.gpsimd if USE_GPS_TAIL else nc.vector
                        eng3.tensor_add(
                            yt[:, 0:ln], yt[:, 0:ln],
                            x_sb[:, ct * NH + a0 : ct * NH + a0 + ln])
                        nc.sync.dma_start(
                            y[ct * 128 : (ct + 1) * 128, a0 : a0 + ln],
                            yt[:, 0:ln])
                    return th

                return ([th_recip] + [norm_pair(hg) for hg in range(4)]
                        + [proj_ct(ct) for ct in range(KT)]
                        + [th_mean, th_var, th_rows]
                        + [y_ct(ct) for ct in range(KT)])

            workq = []
            for cc, (a0, ln) in enumerate(CHUNKS):
                pocs = []
                dent = pden.tile([8, 512], BF, tag="dent", name="dent")
                GSZ = [3, 2, 4, 7]
                for hg in range(HEADS // 2):
                    if ln == 512:
                        run_hg_512(cc, hg, a0, ln, dent, pocs, workq)
                    else:
                        run_hg_128(cc, hg, a0, ln, dent, pocs, workq)
                    for _ in range(GSZ[hg]):
                        if workq:
                            workq.pop(0)()
                workq.extend(tail_phases(cc, a0, ln, dent, pocs))
            for th in workq:
                th()
    _split_multi_waits(nc)
    return nc


def _prep_inputs(x, context, norm_gamma, null_kv, Wq, Wkv, ctx_ln_g, ctx_ln_b,
                 Wctx, bctx, Wout, out_ln_g):
    import ml_dtypes
    bf = ml_dtypes.bfloat16
    f = np.float32
    x = np.asarray(x, f).reshape(4, C, N)
    context = np.asarray(context, f)
    g = np.asarray(norm_gamma, f)
    scale = 1.0 / np.sqrt(DH)
    wq_h = (g[:, None] * np.asarray(Wq, f)) * scale
    negcq_h = -wq_h.sum(0, dtype=np.float64).astype(f)[None, :]
    wkv_h = g[:, None] * np.asarray(Wkv, f)
    # combined stationary: [v | k] so k lands on psum rows 64:128
    wkvc_h = np.concatenate([wkv_h[:, DH:], wkv_h[:, :DH]], axis=1)
    ncs = -wkv_h.sum(0, dtype=np.float64).astype(f)
    ncskv_h = np.concatenate([ncs[DH:], ncs[:DH]])[None, :]
    wctx_h = np.asarray(ctx_ln_g, f)[:, None] * np.asarray(Wctx, f)
    bctx_h = (np.asarray(bctx, f) + np.asarray(ctx_ln_b, f) @ np.asarray(Wctx, f))
    null = np.asarray(null_kv, f)
    W_o = np.asarray(Wout, f)
    # head-pair stacked: rows 0:64 = even head dims, 64:128 = odd head dims
    wout_b = np.concatenate(
        [np.concatenate([W_o[2 * hg * DH : (2 * hg + 1) * DH, :],
                         W_o[(2 * hg + 1) * DH : (2 * hg + 2) * DH, :]], axis=0)
         for hg in range(HEADS // 2)], axis=1)
    # pair selector: block hg, cols 0:64 -> row 2hg, cols 64:128 -> row 2hg+1
    sel_h = np.zeros((8, 4 * 128), f)
    for hg in range(4):
        sel_h[2 * hg, hg * 128 : hg * 128 + 64] = 1.0
        sel_h[2 * hg + 1, hg * 128 + 64 : (hg + 1) * 128] = 1.0

    def mirror(a):
        """[K*128, n] row-major -> SBUF mirror [128, K*n]."""
        K = a.shape[0] // 128
        return np.ascontiguousarray(
            a.reshape(K, 128, -1).transpose(1, 0, 2).reshape(128, -1))

    shared = {
        "wq": np.ascontiguousarray(wq_h).astype(bf),
        "negcq": negcq_h.astype(bf),
        "wkvc": np.ascontiguousarray(wkvc_h).astype(bf),
        "ncskv": np.ascontiguousarray(ncskv_h).astype(bf),
        "wctx": np.ascontiguousarray(wctx_h),
        "bctxk": np.ascontiguousarray(bctx_h[:DH, None]),
        "bctxv": np.ascontiguousarray(bctx_h[DH:, None]),
        "nullkt": np.ascontiguousarray(null[0][:, None]),
        "nullv": np.ascontiguousarray(null[1][:, None]),
        "wout": np.ascontiguousarray(wout_b).astype(bf),
        "selin": sel_h,
        "outgr": np.ascontiguousarray(np.asarray(out_ln_g, f)[None, :]),
    }
    in_maps = []
    for core in range(8):
        b, half = core // 2, core % 2
        m = dict(shared)
        xo = x[b][:, half * NH : (half + 1) * NH]
        xt = x[b][:, (1 - half) * NH : (2 - half) * NH]
        m["x_own"] = np.ascontiguousarray(xo)
        m["xbf"] = np.ascontiguousarray(
            np.concatenate([xo, xt], axis=1)).astype(bf)
        m["ctxt"] = np.ascontiguousarray(context[b])
        in_maps.append(m)
    return in_maps


_LDW_OPT = [False]


def _patch_ldw_opt():
    import concourse.bass_utils as bu
    if getattr(bu, "_ldwopt_patched", False):
        return
    orig = bu.run_command

    def run2(cmd, **kw):
        if _LDW_OPT[0]:
            cmd = [c.replace("--enable-ldw-opt=false", "--enable-ldw-opt=true")
                   for c in cmd]
        return orig(cmd, **kw)

    bu.run_command = run2
    bu._ldwopt_patched = True


def kernel(**inputs):
    from concourse.bass_utils import run_bass_kernel_spmd
    _patch_ldw_opt()

    if "nc" not in _cached:
        _cached["nc"] = _build_bass()
    nc = _cached["nc"]
    in_maps = _prep_inputs(**inputs)
    kw = {}
    if PROFILE:
        import importlib.util

        if "antenv.axon_hooks" not in sys.modules:
            spec = importlib.util.spec_from_file_location(
                "antenv.axon_hooks", "/opt/trn_rl_repo/antenv/axon_hooks.py")
            m = importlib.util.module_from_spec(spec)
            spec.loader.exec_module(m)
            sys.modules["antenv.axon_hooks"] = m
            import antenv

            antenv.axon_hooks = m
        kw = dict(trace=True, tmpdir=PROFILE_DIR)
    res = run_bass_kernel_spmd(nc, in_maps, list(range(8)), **kw)
    _cached["last"] = res
    out = np.empty((4, C, N), np.float32)
    for core in range(8):
        b, half = core // 2, core % 2
        out[b][:, half * NH : (half + 1) * NH] = res.results[core]["y"]
    return out.reshape(4, C, 48, 48)


# revision 70
# speedup vs baseline: 1.0612x; 1.0155x over previous
"""Trainium2 Bass kernel for nn_Attention_LR_65249143160949 (cross-attention block).

Sharding: 8 cores = 4 batches x 2 token-halves (1152 tokens each). Each core
computes k/v for its whole batch (cheap MQA single head, duplicated within the
pair) and q/attention/output for its own tokens. The host permutes tokens so
each core's own rows come first -> identical SPMD program, no collectives.

On-chip layout: features on partitions, tokens on the free axis. LayerNorm is
folded into the projections (pre-scaled weights + rank-1 -colsum*mu term).
Attention runs in sim^T layout (keys on partitions, query tokens free): kT is
rs-scaled so softmax is a plain exp; the denominator comes free as a ones
column (col 64) of the 128-col-padded v stationary (row 64 of the out psum).

Engine balance (the v1 kernel was ACT+PE serialized at ~460us; this one
measures ~320us):
- exp is split ACT (exact, bf16 out) / DVE (Schraudolph bit-trick: i16 =
  round(sim*184.665 + 16250.4) bitcast bf16, ~3% max err on weights).
- all fp32 broadcast/stat matmuls use float32r (1 cyc/row vs 4) or bf16.
- the 24 per-head reciprocals are batched: den rows DMA-gathered to one
  [8, 512] tile, ONE reciprocal per chunk, then per-pair selector matmuls
  (K=8 one-hot-rows, f32r) broadcast 1/den to the right partitions.
- head-PAIR stacking: the odd head's attn.v uses a second v stationary
  (v in cols 64:128, ones at col 32) so its output lands on psum
  partitions 64:128; ot tiles stack both heads -> out-proj runs K=128
  with pair-stacked Wout (half the matmuls), one rb-bcast per pair.
- attn.v lags exp by ALAG=4 j-tiles so DVE-queue latency never stalls PE.
- the tail (recip/norm/proj/LN2/y) is shredded into small thunks drained
  between head-pair passes of the NEXT chunk.
- GPSIMD (cannot touch PSUM) takes SBUF-only elementwise work: big
  memsets, LN2 squares, the residual add.
- v/wkv stationaries are 128-col padded/fused so FWL weight loads stay
  fast; a PE warmup chain pre-ramps the clock during the DMA wait.
- x arrives bf16 from the host, DMA'd in token chunks on the second
  (ACT) hwdge queue so LN1 starts ~12us in; fp32 x (residual only,
  own half only) goes last. Host-mirrored big-line DMA layouts were
  tried and are SLOWER (chip-level power throttle) - keep the small
  rearranged descriptors.

Walrus quirks handled: one sync-wait per TPB instruction (_split_multi_waits),
no custom DVE ops, engine ops must start at partition 0/32/64/96, GPSIMD has
no PSUM access, f32r tiles must be produced by a rounding op (DVE copy), and
row-tile-CONCURRENT matmul pairs must write different PSUM banks (same-bank
concurrent writes hard-crash the device).
"""

import sys

import numpy as np

if "/opt/trn_rl_repo" not in sys.path:
    sys.path.insert(0, "/opt/trn_rl_repo")

C = 512          # channels
N = 2304         # tokens per batch (48*48)
NH = 1152        # tokens per core
HEADS = 8
DH = 64
CTXL = 77
CTXD = 768
JT = 19          # j tiles of 128: 18 img + 1 (ctx 0:77 | null 77 | pad)
JP = JT * 128
CHUNKS = [(0, 512), (512, 512), (1024, 128)]  # (start, len) token chunks
NCH = len(CHUNKS)
KT = 4           # C / 128
EPS = 1e-5

FEXP_S = 184.6649186888274   # 128 / ln(2)
FEXP_C = 16250.4             # 127*128 - 5.6 (minimax-tuned, round-to-nearest)

PROFILE = False
PROFILE_DIR = None

_cached = {}


USE_DVE_EXP = True
USE_GPS_TAIL = True
USE_F32R = True
USE_DENT_DMA = True
ALAG = 5         # attn.v lags exp by this many j-tiles (512-token chunks)


def _exp_engine(cc, hg, jt):
    """Engine for the softmax exp of (chunk cc, head-pair hg, j-tile jt).
    512-token chunks: every 4th j-tile on DVE (bit-trick exp). Returns
    'act' or 'dve'."""
    if not USE_DVE_EXP:
        return "act"
    return "dve" if (jt % 5) == 4 else "act"


def _exp_engine_128(hg, grp):
    """Engine for the packed 4-j-tile exp groups of the 128-token chunk."""
    if not USE_DVE_EXP:
        return "act"
    return "dve" if grp in (1, 3) else "act"


def _split_multi_waits(nc):
    """Walrus codegen supports one sync-wait per TPB instruction (the EVENTS
    struct has a single wait slot). Tile attaches several. Split the extras
    onto same-engine NoOps inserted just before each instruction."""
    import concourse.mybir as mybir

    n = 0
    for fn in nc.m.functions:
        for bb in fn.blocks:
            insts = bb.instructions
            i = 0
            while i < len(insts):
                ins = insts[i]
                si = getattr(ins, "sync_info", None)
                if si is not None and si.on_wait and len(si.on_wait) > 1:
                    waits = list(si.on_wait)
                    for w in waits[:-1]:
                        n += 1
                        nop = mybir.InstNoOp(name=f"WSPLIT-{n}", engine=ins.engine)
                        nop.sync_info = mybir.SyncInfo(on_wait=[w], on_update=[])
                        insts.insert(i, nop)
                        i += 1
                    ins.sync_info = mybir.SyncInfo(
                        on_wait=[waits[-1]], on_update=si.on_update)
                i += 1
    return n


def _build_bass():
    import concourse.bass as bass
    import concourse.mybir as mybir
    import concourse.tile as tile
    from concourse.masks import make_identity
    from contextlib import ExitStack

    F32 = mybir.dt.float32
    F32R = mybir.dt.float32r if USE_F32R else mybir.dt.float32
    BF = mybir.dt.bfloat16
    I16 = mybir.dt.int16
    AF = mybir.ActivationFunctionType
    ALU = mybir.AluOpType

    nc = bass.Bass()
    xbf = nc.declare_dram_parameter("xbf", [C, N], BF, isOutput=False)
    x_own = nc.declare_dram_parameter("x_own", [C, NH], F32, isOutput=False)
    ctxt = nc.declare_dram_parameter("ctxt", [CTXL, CTXD], F32, isOutput=False)
    wq = nc.declare_dram_parameter("wq", [C, C], BF, isOutput=False)
    negcq = nc.declare_dram_parameter("negcq", [1, C], BF, isOutput=False)
    wkvc = nc.declare_dram_parameter("wkvc", [C, 128], BF, isOutput=False)
    ncskv = nc.declare_dram_parameter("ncskv", [1, 128], BF, isOutput=False)
    wctx = nc.declare_dram_parameter("wctx", [CTXD, 2 * DH], F32, isOutput=False)
    bctxk = nc.declare_dram_parameter("bctxk", [DH, 1], F32, isOutput=False)
    bctxv = nc.declare_dram_parameter("bctxv", [DH, 1], F32, isOutput=False)
    nullkt = nc.declare_dram_parameter("nullkt", [DH, 1], F32, isOutput=False)
    nullv = nc.declare_dram_parameter("nullv", [DH, 1], F32, isOutput=False)
    wout = nc.declare_dram_parameter("wout", [128, (HEADS // 2) * C], BF,
                                     isOutput=False)
    selin = nc.declare_dram_parameter("selin", [8, 8 * DH], F32, isOutput=False)
    outgr = nc.declare_dram_parameter("outgr", [1, C], F32, isOutput=False)
    y = nc.declare_dram_parameter("y", [C, NH], F32, isOutput=True)

    with tile.TileContext(nc) as tc, ExitStack() as ctx:
        pconst = ctx.enter_context(tc.tile_pool(name="const", bufs=1))
        pbig = ctx.enter_context(tc.tile_pool(name="big", bufs=1))

        ident = pconst.tile([128, 128], F32)
        make_identity(nc, ident[:])
        ident_bf = pconst.tile([128, 128], BF)
        make_identity(nc, ident_bf[:])
        ones_col = pconst.tile([128, 1], F32)
        nc.vector.memset(ones_col[:], 1.0)
        ones_col_bf = pconst.tile([128, 1], BF)
        nc.vector.memset(ones_col_bf[:], 1.0)
        ones_f = pconst.tile([1, 128], F32)
        nc.vector.memset(ones_f[:], 1.0)
        ones_r = pconst.tile([1, 128], F32R)
        nc.vector.tensor_copy(ones_r[:], ones_f[:])
        eps_col = pconst.tile([128, 1], F32)
        nc.vector.memset(eps_col[:], EPS)
        ones_blk = pconst.tile([128, 64], F32)
        nc.vector.memset(ones_blk[:], 1.0)
        sel_f = pconst.tile([8, 8 * DH], F32)
        sel = pconst.tile([8, 8 * DH], F32R)
        outgr_f = pconst.tile([1, C], F32)
        outgr_r = pconst.tile([1, C], F32R)

        x_sb = pbig.tile([128, KT * NH], F32)        # kt-major; OWN half only
        x_bf = pbig.tile([128, 6 * KT * 384], BF)   # chunk-major: [ch][kt][384] (kept)
        qT = pbig.tile([128, (HEADS // 2) * NH], BF)  # head-pair blocks
        kT2 = pbig.tile([128, JP], BF)               # rs-scaled keys, both halves
        v_sb = pbig.tile([128, JT * 128], BF)        # per j-tile [v 0:64|ones 64|pad]
        v_sb2 = pbig.tile([128, JT * 128], BF)       # odd-head: [0|ones@32|0|v 64:128]
        projBF = pbig.tile([128, KT * 512], BF)      # bf16 proj (per chunk)
        stats = pbig.tile([128, 40], F32)            # col jt: rs_j (v scaling)
        wout_sb = pbig.tile([128, (HEADS // 2) * C], BF)  # head-pair stacked
        # per-token stat rows on partition 0: mu 0:N | rs N:2N
        # (LN2 reuses per cc: mu2 at cc*CH, rs2 at N+cc*CH, ex2 at 2N+cc*CH)
        rows = pbig.tile([1, 2 * N + NH], F32)
        rows_bf = pbig.tile([1, N], BF)
        rows_r = pbig.tile([1, N], F32R)             # f32r copy of rs row
        R_RS, R_SC = N, 2 * N

        nc.sync.dma_start(sel_f[:], selin[:, :])
        nc.sync.dma_start(outgr_f[:], outgr[:, :])
        nc.vector.tensor_copy(sel[:], sel_f[:])
        nc.vector.tensor_copy(outgr_r[:], outgr_f[:])
        nc.sync.dma_start(wout_sb[:], wout[:, :])

        with tc.tile_pool(name="load", bufs=1) as pload, \
             tc.tile_pool(name="x2p", bufs=2) as px2, \
             tc.tile_pool(name="pss", bufs=2, space="PSUM") as pss:
            # psum tags: b1 [<=64,384]x2, bS [128,<=512]x4, bT [128,128]x2
            wq_sb = pload.tile([128, KT * C], BF)
            wkv_sb = pload.tile([128, KT * 128], BF)
            wctx_sb = pload.tile([128, CTXD], F32)
            negcq_sb = pload.tile([1, C], BF)
            ncskv_sb = pload.tile([1, 128], BF)
            bctxk_sb = pload.tile([DH, 1], F32)
            bctxv_sb = pload.tile([DH, 1], F32)
            vT = pload.tile([64, N], BF)
            ck_sb = pload.tile([64, CTXL], F32)
            cv_sb = pload.tile([64, CTXL + 1], F32)
            nullk_st = pload.tile([DH, 1], F32)
            nullv_st = pload.tile([DH, 1], F32)
            ctx_sb = pload.tile([CTXL, CTXD], F32)
            ctxnT = pload.tile([128, 6 * CTXL], F32)
            ex2 = pload.tile([1, N], F32)
            kk = pload.tile([128, 512], F32)

            # PE warmup: ~4us of back-to-back junk matmuls while the DMAs
            # stream in, so the tensor engine reaches its fast pstate
            # before LN1's first real matmul.
            ps_wu = pss.tile([128, 128], F32, tag="bT")
            for _ in range(6):
                nc.tensor.matmul(ps_wu[:, :], ones_f[:], ones_f[:],
                                 start=True, stop=True)

            # x_bf chunks on the ACT queue so LN1 starts while the SP
            # queue streams ctx/weights. SBUF x_bf is chunk-major.
            xbf_d = xbf[:].rearrange("(k p) n -> p k n", p=128)
            xv4 = x_bf[:].rearrange("p (c k n) -> p c k n", c=6, k=KT)
            for ch in range(6):
                a, b = ch * 384, (ch + 1) * 384
                q = nc.scalar if ch % 2 == 0 else nc.sync
                q.dma_start(xv4[:, ch, :, :], xbf_d[:, :, a:b])
            nc.sync.dma_start(ctx_sb[:], ctxt[:, :])
            nc.sync.dma_start(wctx_sb[:].rearrange("p (k n) -> p k n", k=6),
                              wctx[:].rearrange("(k p) n -> p k n", p=128))
            nc.sync.dma_start(bctxk_sb[:], bctxk[:, :])
            nc.sync.dma_start(bctxv_sb[:], bctxv[:, :])
            nc.sync.dma_start(nullk_st[:], nullkt[:, :])
            nc.sync.dma_start(nullv_st[:], nullv[:, :])
            nc.sync.dma_start(wkv_sb[:].rearrange("p (k n) -> p k n", k=KT),
                              wkvc[:].rearrange("(k p) n -> p k n", p=128))
            nc.sync.dma_start(ncskv_sb[:], ncskv[:, :])
            nc.scalar.dma_start(wq_sb[:].rearrange("p (k n) -> p k n", k=KT),
                                wq[:].rearrange("(k p) n -> p k n", p=128))
            nc.sync.dma_start(negcq_sb[:], negcq[:, :])
            x_v = x_sb[:].rearrange("p (k n) -> p k n", k=KT)
            nc.sync.dma_start(x_v[:, :, 0:NH],
                              x_own[:].rearrange("(k p) n -> p k n", p=128))

            # ---- fused per-384-token-chunk pipeline:
            # LN1 stats -> kv projection -> v tiles for the 3 j-tiles,
            # software-pipelined with the x_bf chunk DMAs.
            nc.gpsimd.memset(v_sb[:], 0.0)
            nc.gpsimd.memset(v_sb2[:], 0.0)
            nc.gpsimd.memset(kT2[0:64, 18 * 128 : JP], 0.0)
            for ch in range(6):
                sl = slice(ch * 384, (ch + 1) * 384)
                ps_r1 = pss.tile([64, 384], F32, tag="b1")
                for kt in range(KT):
                    xo = (ch * KT + kt) * 384
                    nc.tensor.matmul(
                        ps_r1[0:1, :], ones_col_bf[:],
                        x_bf[:, xo : xo + 384],
                        start=(kt == 0), stop=(kt == KT - 1))
                nc.scalar.mul(rows[0:1, sl], ps_r1[0:1, :], 1.0 / C)
                nc.vector.tensor_copy(rows_bf[0:1, sl], rows[0:1, sl])
                x2 = px2.tile([128, KT * 384], BF, tag="x2")
                ps_r2 = pss.tile([64, 384], F32, tag="b1")
                for kt in range(KT):
                    xs = x_bf[:, (ch * KT + kt) * 384 : (ch * KT + kt + 1) * 384]
                    nc.vector.tensor_mul(x2[:, kt * 384 : (kt + 1) * 384], xs, xs)
                    nc.tensor.matmul(
                        ps_r2[0:1, :], ones_col_bf[:],
                        x2[:, kt * 384 : (kt + 1) * 384],
                        start=(kt == 0), stop=(kt == KT - 1))
                nc.scalar.mul(ex2[0:1, ch * 384 : (ch + 1) * 384],
                              ps_r2[0:1, :], 1.0 / C)
                a, b = R_RS + ch * 384, R_RS + (ch + 1) * 384
                mu = rows[0:1, ch * 384 : (ch + 1) * 384]
                nc.vector.tensor_mul(rows[0:1, a:b], mu, mu)
                nc.vector.tensor_sub(rows[0:1, a:b],
                                     ex2[0:1, ch * 384 : (ch + 1) * 384],
                                     rows[0:1, a:b])
                nc.scalar.activation(rows[0:1, a:b], rows[0:1, a:b], AF.Ln,
                                     bias=eps_col[0:1, :])
                nc.scalar.activation(rows[0:1, a:b], rows[0:1, a:b], AF.Exp,
                                     scale=-0.5)
                nc.vector.tensor_copy(rows_r[0:1, sl], rows[0:1, a:b])
                # kv projection for this chunk (LN folded; k -> kT2 top half)
                kvl = 384
                ps_kv = pss.tile([128, 512], F32, tag="bS")
                for kt in range(KT):
                    xs = x_bf[:, (ch * KT + kt) * 384 : (ch * KT + kt + 1) * 384]
                    nc.tensor.matmul(ps_kv[:, 0:kvl],
                                     wkv_sb[:, kt * 128 : (kt + 1) * 128],
                                     xs, start=(kt == 0), stop=False)
                nc.tensor.matmul(ps_kv[:, 0:kvl], ncskv_sb[:], rows_bf[0:1, sl],
                                 start=False, stop=True)
                ps_bc = pss.tile([128, 512], F32, tag="bS")
                nc.tensor.matmul(ps_bc[:, 0:kvl], ones_r[0:1, 0:128],
                                 rows_r[0:1, sl])
                nc.vector.tensor_copy(kk[64:128, 0:kvl], ps_kv[64:128, 0:kvl])
                nc.vector.tensor_mul(kT2[64:128, sl], kk[64:128, 0:kvl],
                                     ps_bc[64:128, 0:kvl])
                nc.vector.tensor_copy(vT[:, sl], ps_kv[0:64, 0:kvl])
                # v tiles + rs columns for the 3 j-tiles of this chunk
                for jt in range(3 * ch, 3 * ch + 3):
                    ps_c = pss.tile([128, 128], F32, tag="bT")
                    nc.tensor.matmul(
                        ps_c[:, 0:1],
                        rows[0:1, R_RS + jt * 128 : R_RS + (jt + 1) * 128],
                        ones_col[0:1, :])
                    nc.vector.tensor_copy(stats[:, jt : jt + 1], ps_c[:, 0:1])
                    ps_vt = pss.tile([128, 128], BF, tag="bT")
                    nc.tensor.transpose(ps_vt[:, 0:64],
                                        vT[:, jt * 128 : (jt + 1) * 128],
                                        ident_bf[:64, :64])
                    vb = jt * 128
                    nc.vector.tensor_scalar_mul(v_sb[:, vb : vb + DH],
                                                ps_vt[:, 0:64],
                                                stats[:, jt : jt + 1])
                    nc.vector.memset(v_sb[:, vb + DH : vb + DH + 1], 1.0)
                    nc.vector.tensor_scalar_mul(v_sb2[:, vb + 64 : vb + 128],
                                                ps_vt[:, 0:64],
                                                stats[:, jt : jt + 1])
                    nc.vector.memset(v_sb2[:, vb + 32 : vb + 33], 1.0)

            # ---- context: LN (layout A, bn_stats) + k/v projection ----
            cstat = pload.tile([CTXL, 3, 6], F32)
            for sg in range(3):
                nc.vector.bn_stats(cstat[:, sg, :],
                                   ctx_sb[:, sg * 256 : (sg + 1) * 256])
            cmv = pload.tile([CTXL, 2], F32)
            nc.vector.bn_aggr(cmv[:], cstat[:])
            nc.scalar.activation(cmv[:, 1:2], cmv[:, 1:2], AF.Ln,
                                 bias=eps_col[0:CTXL, :])
            nc.scalar.activation(cmv[:, 1:2], cmv[:, 1:2], AF.Exp, scale=-0.5)
            nc.vector.tensor_scalar(
                out=ctx_sb[:], in0=ctx_sb[:],
                scalar1=cmv[:, 0:1], scalar2=cmv[:, 1:2],
                op0=ALU.subtract, op1=ALU.mult)
            for kt in range(6):
                ps_ct = pss.tile([128, 128], F32, tag="bT")
                nc.tensor.transpose(ps_ct[:, 0:CTXL],
                                    ctx_sb[:, kt * 128 : (kt + 1) * 128],
                                    ident[:CTXL, :CTXL])
                nc.vector.tensor_copy(ctxnT[:, kt * CTXL : (kt + 1) * CTXL],
                                      ps_ct[:, 0:CTXL])
            ps_ck = pss.tile([64, 384], F32, tag="b1")
            ps_cv = pss.tile([64, 384], F32, tag="b1")
            for kt in range(6):
                nc.tensor.matmul(ps_ck[:, 0:CTXL],
                                 wctx_sb[:, kt * 128 : kt * 128 + DH],
                                 ctxnT[:, kt * CTXL : (kt + 1) * CTXL],
                                 start=(kt == 0), stop=(kt == 5))
                nc.tensor.matmul(ps_cv[:, 0:CTXL],
                                 wctx_sb[:, kt * 128 + DH : (kt + 1) * 128],
                                 ctxnT[:, kt * CTXL : (kt + 1) * CTXL],
                                 start=(kt == 0), stop=(kt == 5))
            nc.vector.tensor_scalar_add(ck_sb[:], ps_ck[:, 0:CTXL], bctxk_sb[:])
            nc.vector.tensor_scalar_add(cv_sb[:, 0:CTXL], ps_cv[:, 0:CTXL],
                                        bctxv_sb[:])
            nc.vector.tensor_copy(cv_sb[:, CTXL : CTXL + 1], nullv_st[:])

            # ---- j-tile 18: [ctx 0:77 | null 77 | pad 78:128] ----
            nc.vector.tensor_copy(kT2[0:64, 18 * 128 : 18 * 128 + CTXL], ck_sb[:])
            nc.vector.tensor_copy(kT2[0:64, 18 * 128 + CTXL : 18 * 128 + CTXL + 1],
                                  nullk_st[:])
            VB = 18 * 128
            ps_cvt = pss.tile([128, 128], F32, tag="bT")
            nc.tensor.transpose(ps_cvt[0 : CTXL + 1, 0:64], cv_sb[:],
                                ident[:64, :64])
            nc.vector.tensor_copy(v_sb[0 : CTXL + 1, VB : VB + DH],
                                  ps_cvt[0 : CTXL + 1, 0:64])
            nc.vector.memset(v_sb[0 : CTXL + 1, VB + DH : VB + DH + 1], 1.0)
            nc.vector.tensor_copy(v_sb2[0 : CTXL + 1, VB + 64 : VB + 128],
                                  ps_cvt[0 : CTXL + 1, 0:64])
            nc.vector.memset(v_sb2[0 : CTXL + 1, VB + 32 : VB + 33], 1.0)

            # ---- duplicate kT to partitions 0:64 (sbuf->sbuf DMA) ----
            nc.sync.dma_start(kT2[0:64, 0 : 18 * 128], kT2[64:128, 0 : 18 * 128])
            nc.sync.dma_start(kT2[64:128, 18 * 128 : JP],
                              kT2[0:64, 18 * 128 : JP])

            # ---- q projection (head pairs; LN + 1/sqrt(dh) folded) ----
            for a0, ln in CHUNKS:
                sl = slice(a0, a0 + ln)
                ps_rs = pss.tile([128, 512], F32, tag="bS")
                nc.tensor.matmul(ps_rs[:, 0:ln], ones_r[0:1, :],
                                 rows_r[0:1, sl])
                rs_b = px2.tile([128, 512], F32, tag="rsb")
                nc.vector.tensor_copy(rs_b[:, 0:ln], ps_rs[:, 0:ln])
                for hg in range(HEADS // 2):
                    ps_q = pss.tile([128, 512], F32, tag="bS")
                    for kt in range(KT):
                        nc.tensor.matmul(
                            ps_q[:, 0:ln],
                            wq_sb[:, kt * C + hg * 128 : kt * C + (hg + 1) * 128],
                            x_bf[:, kt * N + a0 : kt * N + a0 + ln],
                            start=(kt == 0), stop=False)
                    nc.tensor.matmul(ps_q[:, 0:ln],
                                     negcq_sb[0:1, hg * 128 : (hg + 1) * 128],
                                     rows_bf[0:1, sl], start=False, stop=True)
                    nc.vector.tensor_mul(
                        qT[:, hg * NH + a0 : hg * NH + a0 + ln],
                        ps_q[:, 0:ln], rs_b[:, 0:ln])

        # ========= attention + output + LN2 + residual, per chunk =========
        # Per (chunk, head-pair): sim pair (row-tiled concurrent) -> exp on
        # ACT or DVE (bit-trick) -> attn.v pair lagging one j-tile. The tail
        # (recip batch, norm, out-proj, LN2, y) is deferred one stage so it
        # never head-of-line-blocks the PE queue.
        with tc.tile_pool(name="attb", bufs=6) as patb, \
             tc.tile_pool(name="atti", bufs=3) as pati, \
             tc.tile_pool(name="outp", bufs=6) as pout, \
             tc.tile_pool(name="pocp", bufs=6) as ppoc, \
             tc.tile_pool(name="denp", bufs=2) as pden, \
             tc.tile_pool(name="recp", bufs=2) as prec, \
             tc.tile_pool(name="rowp", bufs=1) as prow, \
             tc.tile_pool(name="p2p", bufs=1) as pp2, \
             tc.tile_pool(name="yp", bufs=2) as pyt, \
             tc.tile_pool(name="psatt", bufs=2, space="PSUM") as psA, \
             tc.tile_pool(name="psacc", bufs=4, space="PSUM") as psB:

            def at_tile(engine):
                if engine == "act":
                    return patb.tile([128, 1024], BF, tag="atb", name="atb")
                return pati.tile([128, 1024], I16, tag="ati", name="ati")

            def do_exp(engine, at, ps_s, lo, hi):
                if engine == "act":
                    nc.scalar.activation(at[:, lo:hi], ps_s[:, lo:hi], AF.Exp)
                else:
                    nc.vector.tensor_scalar(
                        out=at[:, lo:hi], in0=ps_s[:, lo:hi],
                        scalar1=FEXP_S, scalar2=FEXP_C,
                        op0=ALU.mult, op1=ALU.add)

            def at_slice(at_eng, lo, hi):
                at, eng = at_eng
                ap = at[:, lo:hi]
                return ap if eng == "act" else ap.bitcast(BF)

            def run_hg_512(cc, hg, a0, ln, dent, pocs, workq):
                po0 = psB.tile([128, 512], F32, tag="pop", bufs=2)
                po1 = psB.tile([128, 512], F32, tag="pop", bufs=2)
                po = [po0, po1]
                q0 = qT[0:64, hg * NH + a0 : hg * NH + a0 + ln]
                q1 = qT[64:128, hg * NH + a0 : hg * NH + a0 + ln]
                ats = [None] * JT
                for jt in range(JT):
                    ps_s = psA.tile([128, 1024], F32, tag="sim")
                    nc.tensor.matmul(ps_s[:, 0:ln],
                                     kT2[0:64, jt * 128 : (jt + 1) * 128],
                                     q0, start=True, stop=True)
                    nc.tensor.matmul(ps_s[:, 512 : 512 + ln],
                                     kT2[64:128, jt * 128 : (jt + 1) * 128],
                                     q1, start=True, stop=True)
                    eng = _exp_engine(cc, hg, jt)
                    at = at_tile(eng)
                    do_exp(eng, at, ps_s, 0, 1024)
                    ats[jt] = (at, eng)
                    if jt >= ALAG:
                        j0 = jt - ALAG
                        nc.tensor.matmul(po[0][:, 0:ln],
                                         v_sb[:, j0 * 128 : (j0 + 1) * 128],
                                         at_slice(ats[j0], 0, ln),
                                         start=(j0 == 0), stop=False)
                        nc.tensor.matmul(po[1][:, 0:ln],
                                         v_sb2[:, j0 * 128 : (j0 + 1) * 128],
                                         at_slice(ats[j0], 512, 512 + ln),
                                         start=(j0 == 0), stop=False)
                        ats[j0] = None
                for j0 in range(JT - ALAG, JT):
                    nc.tensor.matmul(po[0][:, 0:ln],
                                     v_sb[:, j0 * 128 : (j0 + 1) * 128],
                                     at_slice(ats[j0], 0, ln),
                                     start=(j0 == 0), stop=(j0 == JT - 1))
                    nc.tensor.matmul(po[1][:, 0:ln],
                                     v_sb2[:, j0 * 128 : (j0 + 1) * 128],
                                     at_slice(ats[j0], 512, 512 + ln),
                                     start=(j0 == 0), stop=(j0 == JT - 1))
                _extract(hg, ln, po, dent, pocs)

            def run_hg_128(cc, hg, a0, ln, dent, pocs, workq):
                # 19 j-tiles packed 4-per-psum-tile; slot g: q0 at col 128g
                # (bank A), q1 at 512+128g (bank B) so the row-tiled
                # concurrent sim pair never co-writes one psum bank.
                po0 = psB.tile([128, 512], F32, tag="pop", bufs=2)
                po1 = psB.tile([128, 512], F32, tag="pop", bufs=2)
                po = [po0, po1]
                q0 = qT[0:64, hg * NH + a0 : hg * NH + a0 + ln]
                q1 = qT[64:128, hg * NH + a0 : hg * NH + a0 + ln]
                GRP = [(0, 4), (4, 4), (8, 4), (12, 4), (16, 3)]
                ats = [None] * len(GRP)
                for gi, (jt0, ng) in enumerate(GRP):
                    ps_s = psA.tile([128, 1024], F32, tag="sim")
                    for g in range(ng):
                        jt = jt0 + g
                        nc.tensor.matmul(
                            ps_s[:, 128 * g : 128 * g + ln],
                            kT2[0:64, jt * 128 : (jt + 1) * 128],
                            q0, start=True, stop=True)
                        nc.tensor.matmul(
                            ps_s[:, 512 + 128 * g : 512 + 128 * g + ln],
                            kT2[64:128, jt * 128 : (jt + 1) * 128],
                            q1, start=True, stop=True)
                    eng = _exp_engine_128(hg, gi)
                    at = at_tile(eng)
                    if ng == 4:
                        do_exp(eng, at, ps_s, 0, 1024)
                    else:
                        do_exp(eng, at, ps_s, 0, 128 * ng)
                        do_exp(eng, at, ps_s, 512, 512 + 128 * ng)
                    ats[gi] = (at, eng)
                    if gi > 0:
                        _attnv_128(po, ats[gi - 1], GRP[gi - 1], ln,
                                   start=(gi == 1), stop=False)
                        ats[gi - 1] = None
                _attnv_128(po, ats[-1], GRP[-1], ln, start=False, stop=True)
                _extract(hg, ln, po, dent, pocs)

            def _attnv_128(po, at_eng, grp, ln, start, stop):
                jt0, ng = grp
                for g in range(ng):
                    jt = jt0 + g
                    last = stop and (g == ng - 1)
                    nc.tensor.matmul(po[0][:, 0:ln],
                                     v_sb[:, jt * 128 : (jt + 1) * 128],
                                     at_slice(at_eng, 128 * g, 128 * g + ln),
                                     start=(start and g == 0), stop=last)
                    nc.tensor.matmul(po[1][:, 0:ln],
                                     v_sb2[:, jt * 128 : (jt + 1) * 128],
                                     at_slice(at_eng, 512 + 128 * g,
                                              512 + 128 * g + ln),
                                     start=(start and g == 0), stop=last)

            def _extract(hg, ln, po, dent, pocs):
                # stacked ot: even head out -> partitions 0:64 (from po0),
                # odd head out -> 64:128 (po1; its v sat cols 64:128).
                # dens: po0 row 64 (even), po1 row 32 (odd).
                otp = ppoc.tile([128, 512], BF, tag="poc")
                nc.vector.tensor_copy(otp[0:64, 0:ln], po[0][0:64, 0:ln])
                nc.vector.tensor_copy(otp[64:128, 0:ln], po[1][64:128, 0:ln])
                stg = ppoc.tile([65, 512], BF, tag="stg")
                nc.vector.tensor_copy(stg[64:65, 0:ln], po[0][64:65, 0:ln])
                nc.vector.tensor_copy(stg[32:33, 0:ln], po[1][32:33, 0:ln])
                nc.sync.dma_start(dent[2 * hg : 2 * hg + 1, 0:ln],
                                  stg[64:65, 0:ln])
                nc.sync.dma_start(dent[2 * hg + 1 : 2 * hg + 2, 0:ln],
                                  stg[32:33, 0:ln])
                pocs.append(otp)

            def tail_phases(cc, a0, ln, dent, pocs):
                ots = []
                rec8r = prec.tile([8, 512], F32R, tag="rec")
                rs2r = prow.tile([1, 2 * 512], F32R, tag="r2")

                def th_recip():
                    dentf = prec.tile([8, 512], F32, tag="dentf")
                    nc.vector.tensor_copy(dentf[:, 0:ln], dent[:, 0:ln])
                    rec8 = prec.tile([8, 512], F32, tag="rec8")
                    nc.vector.reciprocal(rec8[:, 0:ln], dentf[:, 0:ln])
                    nc.vector.tensor_copy(rec8r[:, 0:ln], rec8[:, 0:ln])

                def norm_pair(hg):
                    def th():
                        ps_rb = psB.tile([128, 512], F32, tag="po", bufs=2)
                        nc.tensor.matmul(ps_rb[:, 0:ln],
                                         sel[:, hg * 128 : (hg + 1) * 128],
                                         rec8r[:, 0:ln])
                        ot = pout.tile([128, 512], BF, tag="ot", name="ot")
                        nc.vector.tensor_mul(ot[:, 0:ln], pocs[hg][:, 0:ln],
                                             ps_rb[:, 0:ln])
                        ots.append(ot)
                    return th

                def proj_ct(ct):
                    def th():
                        ps_p = psB.tile([128, 512], F32, tag="po", bufs=2)
                        for hg in range(HEADS // 2):
                            nc.tensor.matmul(
                                ps_p[:, 0:ln],
                                wout_sb[:, hg * C + ct * 128 : hg * C + (ct + 1) * 128],
                                ots[hg][:, 0:ln],
                                start=(hg == 0), stop=(hg == HEADS // 2 - 1))
                        nc.vector.tensor_copy(
                            projBF[:, ct * 512 : ct * 512 + ln],
                            ps_p[:, 0:ln])
                    return th

                def th_mean():
                    ps_m2 = psB.tile([128, 512], F32, tag="po", bufs=2)
                    for ct in range(KT):
                        nc.tensor.matmul(ps_m2[0:1, 0:ln], ones_col_bf[:],
                                         projBF[:, ct * 512 : ct * 512 + ln],
                                         start=(ct == 0), stop=(ct == KT - 1))
                    nc.scalar.mul(rows[0:1, a0 : a0 + ln], ps_m2[0:1, 0:ln],
                                  1.0 / C)

                def th_var():
                    sca, scb = R_SC + a0, R_SC + a0 + ln
                    p2 = pp2.tile([128, KT * 512], BF, tag="p2")
                    ps_q2 = psB.tile([128, 512], F32, tag="po", bufs=2)
                    for ct in range(KT):
                        pslc = projBF[:, ct * 512 : ct * 512 + ln]
                        eng2 = nc.gpsimd if USE_GPS_TAIL else nc.vector
                        eng2.tensor_mul(p2[:, ct * 512 : ct * 512 + ln],
                                        pslc, pslc)
                        nc.tensor.matmul(ps_q2[0:1, 0:ln], ones_col_bf[:],
                                         p2[:, ct * 512 : ct * 512 + ln],
                                         start=(ct == 0), stop=(ct == KT - 1))
                    nc.scalar.mul(rows[0:1, sca:scb], ps_q2[0:1, 0:ln], 1.0 / C)

                def th_rows():
                    ra, rb2 = R_RS + a0, R_RS + a0 + ln
                    sca, scb = R_SC + a0, R_SC + a0 + ln
                    nc.vector.tensor_mul(rows[0:1, ra:rb2],
                                         rows[0:1, a0 : a0 + ln],
                                         rows[0:1, a0 : a0 + ln])
                    nc.vector.tensor_sub(rows[0:1, ra:rb2], rows[0:1, sca:scb],
                                         rows[0:1, ra:rb2])
                    nc.scalar.activation(rows[0:1, ra:rb2], rows[0:1, ra:rb2],
                                         AF.Ln, bias=eps_col[0:1, :])
                    nc.scalar.activation(rows[0:1, ra:rb2], rows[0:1, ra:rb2],
                                         AF.Exp, scale=-0.5)
                    nc.vector.tensor_mul(rows[0:1, sca:scb],
                                         rows[0:1, a0 : a0 + ln],
                                         rows[0:1, ra:rb2])
                    nc.vector.tensor_copy(rs2r[0:1, 0:ln], rows[0:1, ra:rb2])
                    nc.vector.tensor_copy(rs2r[0:1, 512 : 512 + ln],
                                          rows[0:1, sca:scb])

                def y_ct(ct):
                    def th():
                        ps_gb = psB.tile([128, 512], F32, tag="po", bufs=2)
                        gsl = outgr_r[0:1, ct * 128 : (ct + 1) * 128]
                        nc.tensor.matmul(ps_gb[:, 0:ln], gsl, rs2r[0:1, 0:ln])
                        yt = pyt.tile([128, 512], F32, tag="yt", name="yt")
                        pslice = projBF[:, ct * 512 : ct * 512 + ln]
                        nc.vector.tensor_mul(yt[:, 0:ln], pslice, ps_gb[:, 0:ln])
                        ps_gm = psB.tile([128, 512], F32, tag="po", bufs=2)
                        nc.tensor.matmul(ps_gm[:, 0:ln], gsl,
                                         rs2r[0:1, 512 : 512 + ln])
                        nc.vector.tensor_sub(yt[:, 0:ln], yt[:, 0:ln],
                                             ps_gm[:, 0:ln])
                        eng3 = nc.gpsimd if USE_GPS_TAIL else nc.vector
                        eng3.tensor_add(
                            yt[:, 0:ln], yt[:, 0:ln],
                            x_sb[:, ct * NH + a0 : ct * NH + a0 + ln])
                        nc.sync.dma_start(
                            y[ct * 128 : (ct + 1) * 128, a0 : a0 + ln],
                            yt[:, 0:ln])
                    return th

                return ([th_recip] + [norm_pair(hg) for hg in range(4)]
                        + [proj_ct(ct) for ct in range(KT)]
                        + [th_mean, th_var, th_rows]
                        + [y_ct(ct) for ct in range(KT)])

            workq = []
            for cc, (a0, ln) in enumerate(CHUNKS):
                pocs = []
                dent = pden.tile([8, 512], BF, tag="dent", name="dent")
                GSZ = [3, 2, 4, 7]
                for hg in range(HEADS // 2):
                    if ln == 512:
                        run_hg_512(cc, hg, a0, ln, dent, pocs, workq)
                    else:
                        run_hg_128(cc, hg, a0, ln, dent, pocs, workq)
                    for _ in range(GSZ[hg]):
                        if workq:
                            workq.pop(0)()
                workq.extend(tail_phases(cc, a0, ln, dent, pocs))
            for th in workq:
                th()
    _split_multi_waits(nc)
    return nc


def _prep_inputs(x, context, norm_gamma, null_kv, Wq, Wkv, ctx_ln_g, ctx_ln_b,
                 Wctx, bctx, Wout, out_ln_g):
    import ml_dtypes
    bf = ml_dtypes.bfloat16
    f = np.float32
    x = np.asarray(x, f).reshape(4, C, N)
    context = np.asarray(context, f)
    g = np.asarray(norm_gamma, f)
    scale = 1.0 / np.sqrt(DH)
    wq_h = (g[:, None] * np.asarray(Wq, f)) * scale
    negcq_h = -wq_h.sum(0, dtype=np.float64).astype(f)[None, :]
    wkv_h = g[:, None] * np.asarray(Wkv, f)
    # combined stationary: [v | k] so k lands on psum rows 64:128
    wkvc_h = np.concatenate([wkv_h[:, DH:], wkv_h[:, :DH]], axis=1)
    ncs = -wkv_h.sum(0, dtype=np.float64).astype(f)
    ncskv_h = np.concatenate([ncs[DH:], ncs[:DH]])[None, :]
    wctx_h = np.asarray(ctx_ln_g, f)[:, None] * np.asarray(Wctx, f)
    bctx_h = (np.asarray(bctx, f) + np.asarray(ctx_ln_b, f) @ np.asarray(Wctx, f))
    null = np.asarray(null_kv, f)
    W_o = np.asarray(Wout, f)
    # head-pair stacked: rows 0:64 = even head dims, 64:128 = odd head dims
    wout_b = np.concatenate(
        [np.concatenate([W_o[2 * hg * DH : (2 * hg + 1) * DH, :],
                         W_o[(2 * hg + 1) * DH : (2 * hg + 2) * DH, :]], axis=0)
         for hg in range(HEADS // 2)], axis=1)
    # pair selector: block hg, cols 0:64 -> row 2hg, cols 64:128 -> row 2hg+1
    sel_h = np.zeros((8, 4 * 128), f)
    for hg in range(4):
        sel_h[2 * hg, hg * 128 : hg * 128 + 64] = 1.0
        sel_h[2 * hg + 1, hg * 128 + 64 : (hg + 1) * 128] = 1.0

    def mirror(a):
        """[K*128, n] row-major -> SBUF mirror [128, K*n]."""
        K = a.shape[0] // 128
        return np.ascontiguousarray(
            a.reshape(K, 128, -1).transpose(1, 0, 2).reshape(128, -1))

    shared = {
        "wq": np.ascontiguousarray(wq_h).astype(bf),
        "negcq": negcq_h.astype(bf),
        "wkvc": np.ascontiguousarray(wkvc_h).astype(bf),
        "ncskv": np.ascontiguousarray(ncskv_h).astype(bf),
        "wctx": np.ascontiguousarray(wctx_h),
        "bctxk": np.ascontiguousarray(bctx_h[:DH, None]),
        "bctxv": np.ascontiguousarray(bctx_h[DH:, None]),
        "nullkt": np.ascontiguousarray(null[0][:, None]),
        "nullv": np.ascontiguousarray(null[1][:, None]),
        "wout": np.ascontiguousarray(wout_b).astype(bf),
        "selin": sel_h,
        "outgr": np.ascontiguousarray(np.asarray(out_ln_g, f)[None, :]),
    }
    in_maps = []
    for core in range(8):
        b, half = core // 2, core % 2
        m = dict(shared)
        xo = x[b][:, half * NH : (half + 1) * NH]
        xt = x[b][:, (1 - half) * NH : (2 - half) * NH]
        m["x_own"] = np.ascontiguousarray(xo)
        m["xbf"] = np.ascontiguousarray(
            np.concatenate([xo, xt], axis=1)).astype(bf)
        m["ctxt"] = np.ascontiguousarray(context[b])
        in_maps.append(m)
    return in_maps


_LDW_OPT = [False]


def _patch_ldw_opt():
    import concourse.bass_utils as bu
    if getattr(bu, "_ldwopt_patched", False):
        return
    orig = bu.run_command

    def run2(cmd, **kw):
        if _LDW_OPT[0]:
            cmd = [c.replace("--enable-ldw-opt=false", "--enable-ldw-opt=true")
                   for c in cmd]
        return orig(cmd, **kw)

    bu.run_command = run2
    bu._ldwopt_patched = True


def kernel(**inputs):
    from concourse.bass_utils import run_bass_kernel_spmd
    _patch_ldw_opt()

    if "nc" not in _cached:
        _cached["nc"] = _build_bass()
    nc = _cached["nc"]
    in_maps = _prep_inputs(**inputs)
    kw = {}
    if PROFILE:
        import importlib.util

        if "antenv.axon_hooks" not in sys.modules:
            spec = importlib.util.spec_from_file_location(
                "antenv.axon_hooks", "/opt/trn_rl_repo/antenv/axon_hooks.py")
            m = importlib.util.module_from_spec(spec)
            spec.loader.exec_module(m)
            sys.modules["antenv.axon_hooks"] = m
            import antenv

            antenv.axon_hooks = m
        kw = dict(trace=True, tmpdir=PROFILE_DIR)
    res = run_bass_kernel_spmd(nc, in_maps, list(range(8)), **kw)
    _cached["last"] = res
    out = np.empty((4, C, N), np.float32)
    for core in range(8):
        b, half = core // 2, core % 2
        out[b][:, half * NH : (half + 1) * NH] = res.results[core]["y"]
    return out.reshape(4, C, 48, 48)
